# revision 1
# baseline (speedup 1.0000x reference)
"""AGNN (AMNet) message-passing kernel for 8 TRN2 NeuronCores.

Design (vs the v1 baseline):
  - Nodes partitioned contiguously across 8 cores (no global sort).
  - Phase 1: input MLP in bf16, 4 blocks (512 nodes) per iteration; packed
    [hn|h] bf16 rows stored to a local HBM table `agin` in partition-major
    layout (node n -> flat row (n%128)*98 + n//128).
  - Phase 2: ONE AllGather replicates the table (addr_space Shared).
  - Per-chunk independent dst orderings: for each of the 4 table chunks
    (int16 gather reach), each core re-sorts its OWN dsts by that chunk's
    in-degree, making the per-(block,chunk) rectangle schedule tight
    (~1.1x padding vs 2.07x for a common ordering).
  - Device outputs per-chunk partial aggregates (fp32) and exp-sums; the
    softmax combine across chunks + self-loop term + classifier run on the
    HOST (they commute with the chunk decomposition).
  - No tile_critical / manual semaphores: tile auto-tracks dma_gather.

kernel() accepts FULL inputs and returns the FULL [N, 2] float32 output.
"""

import math
import os
import sys

sys.path.insert(0, "/opt/trn_rl_repo")

import numpy as np

CORES = 8
P = 128
NCH = 4
N = 100000
NP = N // CORES            # 12500
NB = math.ceil(NP / P)     # 98
NPP = NB * P               # 12544
CHROWS = 2 * NPP           # 25088
ZROW = NPP - 1             # 12543  == (12543%128)*98 + 12543//128
SBB = 8                    # blocks per super-block (one gather each)
GB = SBB                   # psum->sbuf copy group
IN, HID = 128, 64
D2 = 2 * HID
NIT = NB // 4 + (1 if NB % 4 else 0)  # 25 MLP iterations (4 blocks each)

AGG_DVE = bool(os.environ.get("AGG_DVE"))  # A/B: aggregation on DVE vs PE


def _flatrow(n):
    """Local table row of local node id n (partition-major layout)."""
    return (n % P) * NB + n // P


def _wrap_idx(flat):
    n = flat.shape[0]
    w = flat.reshape(n // 16, 16).T
    return np.tile(w, (8, 1)).astype(np.int16)


# ----------------------------------------------------------------- host prep
def _preprocess(edge_index):
    src = np.asarray(edge_index[0], dtype=np.int64)
    dst = np.asarray(edge_index[1], dtype=np.int64)
    chunk_of_src = src // (2 * NP)

    # CSR sorted by (dst, chunk)
    order = np.lexsort((src, chunk_of_src, dst))
    src_s = src[order]
    ch_s = chunk_of_src[order]
    # per (dst, chunk) counts
    cnt = np.zeros((N, NCH), np.int64)
    np.add.at(cnt, (dst, chunk_of_src), 1)
    cnt_cum = np.concatenate(
        [np.zeros((N, 1), np.int64), np.cumsum(cnt, axis=1)], axis=1
    )  # [N, 5]
    deg = cnt.sum(axis=1)
    row_start = np.zeros(N + 1, np.int64)
    np.cumsum(deg, out=row_start[1:])

    # local table row of each src (within its chunk)
    q = src_s // NP
    nloc = src_s % NP
    locrow = (q % 2) * NPP + (nloc % P) * NB + nloc // P  # within-chunk row

    # per-(core, chunk) orderings + per-block maxima
    pos_pc = np.zeros((CORES, NCH, NP), np.int64)   # node -> position
    Kpc = np.zeros((CORES, NCH, NB), np.int64)
    for p in range(CORES):
        lo = p * NP
        cl = cnt[lo:lo + NP]                         # [NP, NCH]
        for c in range(NCH):
            o = np.argsort(-cl[:, c], kind="stable")
            pos = np.empty(NP, np.int64)
            pos[o] = np.arange(NP)
            pos_pc[p, c] = pos
            srt = cl[o, c]
            padded = np.zeros(NPP, np.int64)
            padded[:NP] = srt
            Kpc[p, c] = padded.reshape(NB, P).max(axis=1)
    K = Kpc.max(axis=0)                              # [NCH, NB]
    K = np.maximum(K, 1)

    # column starts per (c, b) and super-block layout (identical all cores)
    colstart = np.zeros((NCH, NB), np.int64)
    sbs = []  # list of (c, [blocks], col0, ncols)
    coff = 0
    for c in range(NCH):
        for s0 in range(0, NB, SBB):
            blocks = list(range(s0, min(s0 + SBB, NB)))
            c0 = coff
            for b in blocks:
                colstart[c, b] = coff
                coff += int(K[c, b])
            sbs.append((c, blocks, c0, coff - c0))
    TOTC = coff

    # index streams
    idx_all = np.zeros((CORES, P, TOTC * 8), np.int16)
    sidx_all = np.zeros((CORES, NCH, P, NPP // 16), np.int16)
    for p in range(CORES):
        lo = p * NP
        e0, e1 = row_start[lo], row_start[lo + NP]
        d_e = dst[order[e0:e1]] - lo
        c_e = ch_s[e0:e1]
        r_e = locrow[e0:e1]
        # k-rank of each edge within its (dst, chunk) segment
        seg0 = row_start[d_e + lo] - e0 + cnt_cum[d_e + lo, c_e]
        k_e = np.arange(e1 - e0) - seg0
        A = np.full((TOTC, P), ZROW, np.int16)
        pos_e = pos_pc[p, c_e, d_e]
        col_e = colstart[c_e, pos_e // P] + k_e
        A[col_e, pos_e % P] = r_e.astype(np.int16)
        idx_all[p] = _wrap_idx(A.ravel())
        for c in range(NCH):
            o = np.argsort(pos_pc[p, c], kind="stable")  # position -> node
            rows = np.full(NPP, ZROW, np.int64)
            rows[:NP] = _flatrow(o)
            sidx_all[p, c] = _wrap_idx(rows)

    return dict(K=K, TOTC=TOTC, sbs=sbs, colstart=colstart,
                pos_pc=pos_pc, cnt=cnt, idx_all=idx_all, sidx_all=sidx_all)


# ------------------------------------------------------------------ builder
def _patch_walrus_args():
    import concourse.bass_utils as bu
    if getattr(bu, "_agnn_dge_patch", False):
        return
    orig = bu.get_walrus_args

    def patched(*a, **k):
        return list(orig(*a, **k)) + [
            "--dge-levels=io,spill_reload,scalar_dynamic_offset,"
            "vector_dynamic_offsets,dst_reduce,transpose",
        ]

    bu.get_walrus_args = patched
    bu._agnn_dge_patch = True


def _split_multi_waits(nc):
    import bass_rust
    import concourse.mybir as mybir

    cnt = 0
    for func in nc.m.functions:
        for block in func.blocks:
            out = []
            for inst in block.instructions:
                si = inst.sync_info
                cap = 2 if isinstance(inst, mybir.InstEventSemaphore) else 1
                if (si is not None and si.on_wait and len(si.on_wait) > cap
                        and inst.engine is not None):
                    waits = list(si.on_wait)
                    for w in waits[cap:]:
                        cnt += 1
                        nop = mybir.InstNoOp(
                            name=f"wsplit{cnt}", engine=inst.engine, ins=[], outs=[]
                        )
                        nop.sync_info = bass_rust.SyncInfo(on_wait=[w], on_update=[])
                        try:
                            nc.register_instruction(nop, overwrite=True)
                        except Exception:
                            pass
                        out.append(nop)
                    inst.sync_info = bass_rust.SyncInfo(
                        on_wait=waits[:cap], on_update=list(si.on_update or [])
                    )
                out.append(inst)
            try:
                block.instructions = out
            except Exception:
                block.instructions.clear()
                block.instructions.extend(out)
    return cnt


def _build_nc(cfg):
    _patch_walrus_args()
    import concourse.bacc as bacc
    import concourse.mybir as mybir
    import concourse.tile as tile
    from concourse.masks import make_identity

    K = cfg["K"]; TOTC = cfg["TOTC"]; sbs = cfg["sbs"]

    f32 = mybir.dt.float32
    bf16 = mybir.dt.bfloat16

    nc = bacc.Bacc("TRN2", num_devices=CORES, dynamic_dma_scratch_size=65536)

    xT_t = nc.declare_dram_parameter("xT", [IN, NPP], bf16, isOutput=False)
    idx_t = nc.declare_dram_parameter("idx", [P, TOTC * 8], mybir.dt.int16, isOutput=False)
    sidx_t = nc.declare_dram_parameter("sidx", [NCH, P, NPP // 16], mybir.dt.int16, isOutput=False)
    W1_t = nc.declare_dram_parameter("W1", [IN, HID], bf16, isOutput=False)
    b1_t = nc.declare_dram_parameter("b1", [HID, 1], f32, isOutput=False)
    W2_t = nc.declare_dram_parameter("W2", [HID, HID], bf16, isOutput=False)
    b2_t = nc.declare_dram_parameter("b2", [HID, 1], f32, isOutput=False)
    beta_t = nc.declare_dram_parameter("beta128", [P, 1], f32, isOutput=False)
    agg_t = nc.declare_dram_parameter("agg", [NCH, P, NB, HID], f32, isOutput=True)
    s_t = nc.declare_dram_parameter("s", [NCH, P, NB], f32, isOutput=True)
    sagg_t = nc.declare_dram_parameter("selfagg", [P, NB, HID], f32, isOutput=True)

    with tile.TileContext(nc) as tc:
        agin, free_agin = tc.tile([P, NB, D2], bf16, space="DRAM", name="agin")
        import contextlib
        _tabctx = contextlib.ExitStack()
        _tabpool = _tabctx.enter_context(
            tc.tile_pool(name="tabpool", space="DRAM", bufs=1))
        table = _tabpool.tile([CORES * NPP, D2], bf16, name="table",
                              addr_space="Shared")

        def free_table():
            _tabctx.close()

        with tc.tile_pool(name="consts", bufs=1) as cpool:
            W1sb = cpool.tile([IN, HID], bf16)
            W2sb = cpool.tile([HID, HID], bf16)
            b1sb = cpool.tile([HID, 1], f32)
            b2sb = cpool.tile([HID, 1], f32)
            betasb = cpool.tile([P, 1], f32)
            sbeta = cpool.tile([P, 1], f32)
            ident = cpool.tile([P, P], f32)
            identb = cpool.tile([P, P], bf16)
            hTD = cpool.tile([P, NB, HID], bf16)
            zrow = cpool.tile([1, 1, D2], bf16)
            hnD = [cpool.tile([P, NB, HID], bf16, name=f"hnD{c}") for c in range(NCH)]

            nc.sync.dma_start(out=W1sb[:], in_=W1_t[:])
            nc.sync.dma_start(out=W2sb[:], in_=W2_t[:])
            nc.sync.dma_start(out=b1sb[:], in_=b1_t[:])
            nc.sync.dma_start(out=b2sb[:], in_=b2_t[:])
            nc.sync.dma_start(out=betasb[:], in_=beta_t[:])
            make_identity(nc, ident[:])
            nc.vector.tensor_copy(identb[:], ident[:])
            nc.scalar.activation(sbeta[:], betasb[:], mybir.ActivationFunctionType.Exp)
            nc.gpsimd.memset(zrow[:], 0)
            stile = [cpool.tile([P, NB], f32, name=f"stile{c}") for c in range(NCH)]

            # ---------------- phase 1: MLP + packed local table ------------
            with (
                tc.tile_pool(name="mlp", bufs=3) as mpool,
                tc.tile_pool(name="mlpp", bufs=2, space="PSUM") as mpp,
            ):
                for it in range(NIT):
                    nbk = min(4, NB - 4 * it)
                    cols = nbk * P
                    s0 = it * 4 * P
                    xt = mpool.tile([IN, 512], bf16, tag="xt")
                    nc.sync.dma_start(out=xt[:, 0:cols], in_=xT_t[:, s0:s0 + cols])
                    ps1 = mpp.tile([HID, 512], f32, tag="ps1")
                    nc.tensor.matmul(ps1[:, 0:cols], lhsT=W1sb[:], rhs=xt[:, 0:cols],
                                     start=True, stop=True)
                    r1 = mpool.tile([HID, 512], bf16, tag="r1")
                    nc.scalar.activation(r1[:, 0:cols], ps1[:, 0:cols],
                                         mybir.ActivationFunctionType.Relu, bias=b1sb[:])
                    ps2 = mpp.tile([HID, 512], f32, tag="ps2")
                    nc.tensor.matmul(ps2[:, 0:cols], lhsT=W2sb[:], rhs=r1[:, 0:cols],
                                     start=True, stop=True)
                    hb = mpool.tile([HID, 512], bf16, tag="hb")
                    nc.scalar.activation(hb[:, 0:cols], ps2[:, 0:cols],
                                         mybir.ActivationFunctionType.Identity, bias=b2sb[:])
                    pst = mpp.tile([P, 4, HID], bf16, tag="pst")
                    for j in range(nbk):
                        nc.tensor.transpose(pst[:, j, :], hb[:, j * P:(j + 1) * P],
                                            identb[0:HID, 0:HID])
                    hts = hTD[:, 4 * it:4 * it + nbk, :]
                    nc.scalar.activation(hts, pst[:, 0:nbk, :],
                                         mybir.ActivationFunctionType.Copy)
                    sq = mpool.tile([P, 4, HID], bf16, tag="sq")
                    nc.vector.tensor_tensor(out=sq[:, 0:nbk, :], in0=hts, in1=hts,
                                            op=mybir.AluOpType.mult)
                    ss = mpool.tile([P, 4], f32, tag="ss")
                    nc.vector.tensor_reduce(out=ss[:, 0:nbk], in_=sq[:, 0:nbk, :],
                                            axis=mybir.AxisListType.X,
                                            op=mybir.AluOpType.add)
                    nrm = mpool.tile([P, 4], f32, tag="nrm")
                    nc.scalar.activation(nrm[:, 0:nbk], ss[:, 0:nbk],
                                         mybir.ActivationFunctionType.Sqrt)
                    nc.vector.tensor_scalar_max(out=nrm[:, 0:nbk], in0=nrm[:, 0:nbk],
                                                scalar1=1e-12)
                    rn = mpool.tile([P, 4], f32, tag="rn")
                    nc.vector.reciprocal(rn[:, 0:nbk], nrm[:, 0:nbk])
                    tabt = mpool.tile([P, 4, D2], bf16, tag="tabt")
                    nc.vector.tensor_tensor(
                        out=tabt[:, 0:nbk, 0:HID], in0=hts,
                        in1=rn[:, 0:nbk].unsqueeze(2).to_broadcast([P, nbk, HID]),
                        op=mybir.AluOpType.mult)
                    nc.vector.tensor_copy(tabt[:, 0:nbk, HID:D2], hts)
                    nc.sync.dma_start(out=agin[:, 4 * it:4 * it + nbk, :],
                                      in_=tabt[:, 0:nbk, :])
            # zero row for gather pads — AFTER the MLP stores (overwrites the
            # fake node at (p=127, b=97) = flat row 12543)
            nc.sync.dma_start(out=agin[P - 1:P, NB - 1:NB, :], in_=zrow[:])

            # self-loop partial (base order): exp(beta) * h
            with tc.tile_pool(name="selfp", bufs=1) as spool_:
                sagg = spool_.tile([P, NB, HID], f32)
                nc.vector.tensor_scalar_mul(out=sagg[:], in0=hTD[:], scalar1=sbeta[:])
                nc.sync.dma_start(out=sagg_t[:], in_=sagg[:])

            # ---------------- phase 2: AllGather the table -----------------
            nc.gpsimd.collective_compute(
                "AllGather",
                mybir.AluOpType.bypass,
                replica_groups=[list(range(CORES))],
                ins=[agin[:].flatten_outer_dims().opt()],
                outs=[table[:].opt()],
            )

            # ---------------- phase 2.5: per-chunk dst tables --------------
            with tc.tile_pool(name="hnprep", bufs=1) as hpool:
                aginflat = agin[:].flatten_outer_dims()
                for c in range(NCH):
                    sxi = hpool.tile([P, NPP // 16], mybir.dt.int16, tag="sxi")
                    nc.sync.dma_start(out=sxi[:], in_=sidx_t[c])
                    scr = hpool.tile([P, NB, D2], bf16, tag="scr")
                    nc.gpsimd.dma_gather(scr[:], aginflat, sxi[:], NPP, NPP, D2,
                                         single_packet=False)
                    nc.vector.tensor_copy(hnD[c][:], scr[:, :, 0:HID])

            # ---------------- phase 3: per-chunk AGNN partials -------------
            with (
                tc.tile_pool(name="blk", bufs=3) as bpool,
                tc.tile_pool(name="blks", bufs=3) as spool,
                tc.tile_pool(name="sacc", bufs=2) as apool,
                tc.tile_pool(name="blkp", bufs=2, space="PSUM") as bpp,
            ):
                for (c, blocks, col0, scols) in sbs:
                    wc0, wcn = col0 * 8, scols * 8
                    idxsb = bpool.tile([P, wcn], mybir.dt.int16, tag="idx")
                    nc.sync.dma_start(out=idxsb[:], in_=idx_t[:, wc0:wc0 + wcn])
                    slots = bpool.tile([P, scols, D2], bf16, tag="slots")
                    nidx = P * scols
                    nc.gpsimd.dma_gather(
                        slots[:], table[c * CHROWS:(c + 1) * CHROWS, :], idxsb[:],
                        nidx, nidx, D2, single_packet=False)

                    pgrp = bpp.tile([P, SBB, HID], f32, tag="pgrp")
                    soff = 0
                    for gi, b in enumerate(blocks):
                        Kb = int(K[c, b])
                        sl = slots[:, soff:soff + Kb, :]
                        tmp = spool.tile([P, Kb, HID], bf16, tag="tmp")
                        nc.vector.tensor_tensor(
                            out=tmp[:], in0=sl[:, :, 0:HID],
                            in1=hnD[c][:, b, :].unsqueeze(1).to_broadcast([P, Kb, HID]),
                            op=mybir.AluOpType.mult)
                        delta = spool.tile([P, Kb], f32, tag="delta")
                        nc.vector.tensor_reduce(out=delta[:], in_=tmp[:],
                                                axis=mybir.AxisListType.X,
                                                op=mybir.AluOpType.add)
                        ex = spool.tile([P, Kb], bf16, tag="ex")
                        nc.scalar.activation(ex[:], delta[:],
                                             mybir.ActivationFunctionType.Exp,
                                             scale=betasb[:],
                                             accum_out=stile[c][:, b:b + 1])
                        if AGG_DVE:
                            wtsT = spool.tile([P, HID, Kb], bf16, tag="wtsT")
                            nc.vector.tensor_tensor(
                                out=wtsT[:].transpose([0, 2, 1]),
                                in0=sl[:, :, HID:D2],
                                in1=ex[:].unsqueeze(2).to_broadcast([P, Kb, HID]),
                                op=mybir.AluOpType.mult)
                            nc.vector.tensor_reduce(
                                out=pgrp[:, gi, :], in_=wtsT[:],
                                axis=mybir.AxisListType.X, op=mybir.AluOpType.add)
                        else:
                            wts = spool.tile([P, Kb, HID], bf16, tag="wts")
                            nc.vector.tensor_tensor(
                                out=wts[:], in0=sl[:, :, HID:D2],
                                in1=ex[:].unsqueeze(2).to_broadcast([P, Kb, HID]),
                                op=mybir.AluOpType.mult)
                            for k in range(Kb):
                                nc.tensor.matmul(pgrp[:, gi, :], lhsT=identb[:],
                                                 rhs=wts[:, k, :],
                                                 start=(k == 0), stop=(k == Kb - 1))
                        soff += Kb
                    nbk = len(blocks)
                    aggsb = apool.tile([P, SBB, HID], f32, tag="aggsb")
                    nc.scalar.activation(aggsb[:, 0:nbk, :], pgrp[:, 0:nbk, :],
                                         mybir.ActivationFunctionType.Copy)
                    nc.sync.dma_start(
                        out=agg_t[c, :, blocks[0]:blocks[0] + nbk, :],
                        in_=aggsb[:, 0:nbk, :])
                for c in range(NCH):
                    nc.sync.dma_start(out=s_t[c], in_=stile[c][:])

        free_agin()
        free_table()

    nc.compile()
    _split_multi_waits(nc)
    return nc


# ------------------------------------------------------------------- driver
_CACHE = {}


def _get_nc(cfg_key, cfg):
    if cfg_key not in _CACHE:
        _CACHE[cfg_key] = _build_nc(cfg)
    return _CACHE[cfg_key]


def _make_in_maps(inputs, prep):
    x = np.asarray(inputs["x"], dtype=np.float32)
    W1 = np.asarray(inputs["W1"], dtype=np.float32)
    b1 = np.asarray(inputs["b1"], dtype=np.float32).reshape(-1, 1)
    W2 = np.asarray(inputs["W2"], dtype=np.float32)
    b2 = np.asarray(inputs["b2"], dtype=np.float32).reshape(-1, 1)
    beta = np.asarray(inputs["beta"], dtype=np.float32)
    beta128 = np.repeat(beta.reshape(1, 1), P, axis=0).astype(np.float32)
    import ml_dtypes
    bf = ml_dtypes.bfloat16

    in_maps = []
    for p in range(CORES):
        xp = x[p * NP:(p + 1) * NP]
        xpad = np.zeros((NPP, IN), np.float32)
        xpad[:NP] = xp
        in_maps.append({
            "xT": np.ascontiguousarray(xpad.T).astype(bf),
            "idx": prep["idx_all"][p],
            "sidx": prep["sidx_all"][p],
            "W1": W1.astype(bf), "b1": b1,
            "W2": W2.astype(bf), "b2": b2,
            "beta128": beta128,
        })
    return in_maps


def _postprocess(results, prep, inputs):
    Wc = np.asarray(inputs["Wc"], dtype=np.float64)
    bc = np.asarray(inputs["bc"], dtype=np.float64)
    beta = float(np.asarray(inputs["beta"]).reshape(-1)[0])
    expb = math.exp(beta)
    K = prep["K"]
    y = np.zeros((N, Wc.shape[1]), np.float32)
    n = np.arange(NP)
    bb, dd = n // P, n % P
    for p in range(CORES):
        res = results[p]
        agg = res["agg"].astype(np.float64)       # [4, 128, 98, 64]
        sdev = res["s"].astype(np.float64)        # [4, 128, 98]
        sagg = res["selfagg"].astype(np.float64)  # [128, 98, 64]
        acc = sagg[dd, bb, :].copy()              # self term, base order
        stot = np.full(NP, expb)
        cntp = prep["cnt"][p * NP:(p + 1) * NP]
        for c in range(NCH):
            pos = prep["pos_pc"][p, c]
            b_, d_ = pos // P, pos % P
            acc += agg[c, d_, b_, :]
            stot += sdev[c, d_, b_] - (K[c, b_] - cntp[:, c])
        y[p * NP:(p + 1) * NP] = ((acc / stot[:, None]) @ Wc + bc).astype(np.float32)
    return y


def get_nc_for_test(inputs):
    prep = _preprocess(np.asarray(inputs["edge_index"]))
    cfg = dict(K=prep["K"], TOTC=prep["TOTC"], sbs=prep["sbs"])
    cfg_key = ("v2", AGG_DVE, tuple(int(k) for k in prep["K"].ravel()))
    return _get_nc(cfg_key, cfg)


def kernel(**inputs):
    from concourse.bass_utils import run_bass_kernel_spmd

    prep = _preprocess(np.asarray(inputs["edge_index"]))
    cfg = dict(K=prep["K"], TOTC=prep["TOTC"], sbs=prep["sbs"])
    cfg_key = ("v2", AGG_DVE, tuple(int(k) for k in prep["K"].ravel()))
    nc = _get_nc(cfg_key, cfg)
    in_maps = _make_in_maps(inputs, prep)
    res = run_bass_kernel_spmd(nc, in_maps, core_ids=list(range(CORES)))
    return _postprocess(res.results, prep, inputs)


if __name__ == "__main__":
    pass



# revision 4
# speedup vs baseline: 1.0015x; 1.0015x over previous
"""AGNN (AMNet) message-passing kernel for 8 TRN2 NeuronCores.

Design (vs the v1 baseline):
  - Nodes partitioned contiguously across 8 cores (no global sort).
  - Phase 1: input MLP in bf16, 4 blocks (512 nodes) per iteration; packed
    [hn|h] bf16 rows stored to a local HBM table `agin` in partition-major
    layout (node n -> flat row (n%128)*98 + n//128).
  - Phase 2: ONE AllGather replicates the table (addr_space Shared).
  - Per-chunk independent dst orderings: for each of the 4 table chunks
    (int16 gather reach), each core re-sorts its OWN dsts by that chunk's
    in-degree, making the per-(block,chunk) rectangle schedule tight
    (~1.1x padding vs 2.07x for a common ordering).
  - Device outputs per-chunk partial aggregates (fp32) and exp-sums; the
    softmax combine across chunks + self-loop term + classifier run on the
    HOST (they commute with the chunk decomposition).
  - No tile_critical / manual semaphores: tile auto-tracks dma_gather.

kernel() accepts FULL inputs and returns the FULL [N, 2] float32 output.
"""

import math
import os
import sys

sys.path.insert(0, "/opt/trn_rl_repo")

import numpy as np

CORES = 8
P = 128
NCH = 4
N = 100000
NP = N // CORES            # 12500
NB = math.ceil(NP / P)     # 98
NPP = NB * P               # 12544
CHROWS = 2 * NPP           # 25088
ZROW = NPP - 1             # 12543  == (12543%128)*98 + 12543//128
SBB = 8                    # blocks per super-block (one gather each)
GB = SBB                   # psum->sbuf copy group
IN, HID = 128, 64
D2 = 2 * HID
NIT = NB // 4 + (1 if NB % 4 else 0)  # 25 MLP iterations (4 blocks each)

AGG_DVE = bool(os.environ.get("AGG_DVE"))  # A/B: aggregation on DVE vs PE
MAXG = 8                       # max blocks per compute group (PSUM)
MAXGC = 64                     # max columns per compute group (SBUF)
BATCH_COLS = 64                # gather batch size (columns)
CCOL = 330.0                   # DP cost per padded column (ns-ish)
CGRP = 1500.0                  # DP cost per extra group


def _dp_groups(Kc):
    """Consecutive groups (<=MAXG blocks, <=MAXGC cols) minimizing
    sum(len*Kmax*CCOL + CGRP)."""
    nb = len(Kc)
    best = [float("inf")] * (nb + 1)
    prev = [0] * (nb + 1)
    best[0] = 0.0
    for i in range(1, nb + 1):
        kmax = 0
        for L in range(1, MAXG + 1):
            j = i - L
            if j < 0:
                break
            kmax = max(kmax, Kc[j])
            if L * kmax > MAXGC:
                break
            cst = best[j] + L * kmax * CCOL + CGRP
            if cst < best[i]:
                best[i] = cst
                prev[i] = j
    out = []
    i = nb
    while i > 0:
        j = prev[i]
        out.append((j, i - j, int(max(Kc[j:i]))))
        i = j
    return out[::-1]


def _flatrow(n):
    """Local table row of local node id n (partition-major layout)."""
    return (n % P) * NB + n // P


def _wrap_idx(flat):
    n = flat.shape[0]
    w = flat.reshape(n // 16, 16).T
    return np.tile(w, (8, 1)).astype(np.int16)


# ----------------------------------------------------------------- host prep
def _preprocess(edge_index):
    src = np.asarray(edge_index[0], dtype=np.int64)
    dst = np.asarray(edge_index[1], dtype=np.int64)
    chunk_of_src = src // (2 * NP)

    # CSR sorted by (dst, chunk)
    order = np.lexsort((src, chunk_of_src, dst))
    src_s = src[order]
    ch_s = chunk_of_src[order]
    # per (dst, chunk) counts
    cnt = np.zeros((N, NCH), np.int64)
    np.add.at(cnt, (dst, chunk_of_src), 1)
    cnt_cum = np.concatenate(
        [np.zeros((N, 1), np.int64), np.cumsum(cnt, axis=1)], axis=1
    )  # [N, 5]
    deg = cnt.sum(axis=1)
    row_start = np.zeros(N + 1, np.int64)
    np.cumsum(deg, out=row_start[1:])

    # local table row of each src (within its chunk)
    q = src_s // NP
    nloc = src_s % NP
    locrow = (q % 2) * NPP + (nloc % P) * NB + nloc // P  # within-chunk row

    # per-(core, chunk) orderings + per-block maxima
    pos_pc = np.zeros((CORES, NCH, NP), np.int64)   # node -> position
    Kpc = np.zeros((CORES, NCH, NB), np.int64)
    for p in range(CORES):
        lo = p * NP
        cl = cnt[lo:lo + NP]                         # [NP, NCH]
        for c in range(NCH):
            o = np.argsort(-cl[:, c], kind="stable")
            pos = np.empty(NP, np.int64)
            pos[o] = np.arange(NP)
            pos_pc[p, c] = pos
            srt = cl[o, c]
            padded = np.zeros(NPP, np.int64)
            padded[:NP] = srt
            Kpc[p, c] = padded.reshape(NB, P).max(axis=1)
    K = Kpc.max(axis=0)                              # [NCH, NB]
    K = np.maximum(K, 1)

    # flexible uniform-K groups per chunk + gather batches of whole groups
    colstart = np.zeros((NCH, NB), np.int64)
    Keff = np.zeros((NCH, NB), np.int64)
    groups = []   # [NCH] list of (b0, nbk, Kg, col0)  (col0 global)
    batches = []  # [NCH] list of (col0, ncols, g0, ng)
    coff = 0
    for c in range(NCH):
        gs = _dp_groups(list(K[c]))
        glist = []
        bl = []
        bc0, bg0, bnc = coff, 0, 0
        for gi, (b0, nbk, Kg) in enumerate(gs):
            w = nbk * Kg
            if bnc + w > BATCH_COLS and bnc > 0:
                bl.append((bc0, bnc, bg0, gi - bg0))
                bc0, bg0, bnc = coff, gi, 0
            glist.append((b0, nbk, Kg, coff))
            Keff[c, b0:b0 + nbk] = Kg
            for j in range(nbk):
                colstart[c, b0 + j] = coff + j * Kg
            coff += w
            bnc += w
        bl.append((bc0, bnc, bg0, len(gs) - bg0))
        groups.append(glist)
        batches.append(bl)
    TOTC = coff

    # index streams
    idx_all = np.zeros((CORES, P, TOTC * 8), np.int16)
    sidx_all = np.zeros((CORES, NCH, P, NPP // 16), np.int16)
    for p in range(CORES):
        lo = p * NP
        e0, e1 = row_start[lo], row_start[lo + NP]
        d_e = dst[order[e0:e1]] - lo
        c_e = ch_s[e0:e1]
        r_e = locrow[e0:e1]
        # k-rank of each edge within its (dst, chunk) segment
        seg0 = row_start[d_e + lo] - e0 + cnt_cum[d_e + lo, c_e]
        k_e = np.arange(e1 - e0) - seg0
        A = np.full((TOTC, P), ZROW, np.int16)
        pos_e = pos_pc[p, c_e, d_e]
        col_e = colstart[c_e, pos_e // P] + k_e
        A[col_e, pos_e % P] = r_e.astype(np.int16)
        idx_all[p] = _wrap_idx(A.ravel())
        for c in range(NCH):
            o = np.argsort(pos_pc[p, c], kind="stable")  # position -> node
            rows = np.full(NPP, ZROW, np.int64)
            rows[:NP] = _flatrow(o)
            sidx_all[p, c] = _wrap_idx(rows)

    return dict(K=K, Keff=Keff, TOTC=TOTC, groups=groups, batches=batches,
                colstart=colstart,
                pos_pc=pos_pc, cnt=cnt, idx_all=idx_all, sidx_all=sidx_all)


# ------------------------------------------------------------------ builder
def _patch_walrus_args():
    import concourse.bass_utils as bu
    if getattr(bu, "_agnn_dge_patch", False):
        return
    orig = bu.get_walrus_args

    def patched(*a, **k):
        return list(orig(*a, **k)) + [
            "--dge-levels=io,spill_reload,scalar_dynamic_offset,"
            "vector_dynamic_offsets,dst_reduce,transpose",
        ]

    bu.get_walrus_args = patched
    bu._agnn_dge_patch = True


def _split_multi_waits(nc):
    import bass_rust
    import concourse.mybir as mybir

    cnt = 0
    for func in nc.m.functions:
        for block in func.blocks:
            out = []
            for inst in block.instructions:
                si = inst.sync_info
                cap = 2 if isinstance(inst, mybir.InstEventSemaphore) else 1
                if (si is not None and si.on_wait and len(si.on_wait) > cap
                        and inst.engine is not None):
                    waits = list(si.on_wait)
                    for w in waits[cap:]:
                        cnt += 1
                        nop = mybir.InstNoOp(
                            name=f"wsplit{cnt}", engine=inst.engine, ins=[], outs=[]
                        )
                        nop.sync_info = bass_rust.SyncInfo(on_wait=[w], on_update=[])
                        try:
                            nc.register_instruction(nop, overwrite=True)
                        except Exception:
                            pass
                        out.append(nop)
                    inst.sync_info = bass_rust.SyncInfo(
                        on_wait=waits[:cap], on_update=list(si.on_update or [])
                    )
                out.append(inst)
            try:
                block.instructions = out
            except Exception:
                block.instructions.clear()
                block.instructions.extend(out)
    return cnt


def _build_nc(cfg):
    _patch_walrus_args()
    import concourse.bacc as bacc
    import concourse.mybir as mybir
    import concourse.tile as tile
    from concourse.masks import make_identity

    K = cfg["K"]; TOTC = cfg["TOTC"]
    groups = cfg["groups"]; batches = cfg["batches"]

    f32 = mybir.dt.float32
    bf16 = mybir.dt.bfloat16

    nc = bacc.Bacc("TRN2", num_devices=CORES, dynamic_dma_scratch_size=65536)

    xT_t = nc.declare_dram_parameter("xT", [IN, NPP], bf16, isOutput=False)
    idx_t = nc.declare_dram_parameter("idx", [P, TOTC * 8], mybir.dt.int16, isOutput=False)
    sidx_t = nc.declare_dram_parameter("sidx", [NCH, P, NPP // 16], mybir.dt.int16, isOutput=False)
    W1_t = nc.declare_dram_parameter("W1", [IN, HID], bf16, isOutput=False)
    b1_t = nc.declare_dram_parameter("b1", [HID, 1], f32, isOutput=False)
    W2_t = nc.declare_dram_parameter("W2", [HID, HID], bf16, isOutput=False)
    b2_t = nc.declare_dram_parameter("b2", [HID, 1], f32, isOutput=False)
    beta_t = nc.declare_dram_parameter("beta128", [P, 1], f32, isOutput=False)
    agg_t = nc.declare_dram_parameter("agg", [NCH, P, NB, HID], f32, isOutput=True)
    s_t = nc.declare_dram_parameter("s", [NCH, P, NB], f32, isOutput=True)
    sagg_t = nc.declare_dram_parameter("selfagg", [P, NB, HID], f32, isOutput=True)

    with tile.TileContext(nc) as tc:
        agin, free_agin = tc.tile([P, NB, D2], bf16, space="DRAM", name="agin")
        import contextlib
        _tabctx = contextlib.ExitStack()
        _tabpool = _tabctx.enter_context(
            tc.tile_pool(name="tabpool", space="DRAM", bufs=1))
        table = _tabpool.tile([CORES * NPP, D2], bf16, name="table",
                              addr_space="Shared")

        def free_table():
            _tabctx.close()

        with tc.tile_pool(name="consts", bufs=1) as cpool:
            W1sb = cpool.tile([IN, HID], bf16)
            W2sb = cpool.tile([HID, HID], bf16)
            b1sb = cpool.tile([HID, 1], f32)
            b2sb = cpool.tile([HID, 1], f32)
            betasb = cpool.tile([P, 1], f32)
            sbeta = cpool.tile([P, 1], f32)
            ident = cpool.tile([P, P], f32)
            identb = cpool.tile([P, P], bf16)
            hTD = cpool.tile([P, NB, HID], bf16)
            zrow = cpool.tile([1, 1, D2], bf16)
            hnD = [cpool.tile([P, NB, HID], bf16, name=f"hnD{c}") for c in range(NCH)]

            nc.sync.dma_start(out=W1sb[:], in_=W1_t[:])
            nc.sync.dma_start(out=W2sb[:], in_=W2_t[:])
            nc.sync.dma_start(out=b1sb[:], in_=b1_t[:])
            nc.sync.dma_start(out=b2sb[:], in_=b2_t[:])
            nc.sync.dma_start(out=betasb[:], in_=beta_t[:])
            make_identity(nc, ident[:])
            nc.vector.tensor_copy(identb[:], ident[:])
            nc.scalar.activation(sbeta[:], betasb[:], mybir.ActivationFunctionType.Exp)
            nc.gpsimd.memset(zrow[:], 0)
            stile = [cpool.tile([P, NB], f32, name=f"stile{c}") for c in range(NCH)]

            # ---------------- phase 1: MLP + packed local table ------------
            with (
                tc.tile_pool(name="mlp", bufs=3) as mpool,
                tc.tile_pool(name="mlpp", bufs=2, space="PSUM") as mpp,
            ):
                for it in range(NIT):
                    nbk = min(4, NB - 4 * it)
                    cols = nbk * P
                    s0 = it * 4 * P
                    xt = mpool.tile([IN, 512], bf16, tag="xt")
                    nc.sync.dma_start(out=xt[:, 0:cols], in_=xT_t[:, s0:s0 + cols])
                    ps1 = mpp.tile([HID, 512], f32, tag="ps1")
                    nc.tensor.matmul(ps1[:, 0:cols], lhsT=W1sb[:], rhs=xt[:, 0:cols],
                                     start=True, stop=True)
                    r1 = mpool.tile([HID, 512], bf16, tag="r1")
                    nc.scalar.activation(r1[:, 0:cols], ps1[:, 0:cols],
                                         mybir.ActivationFunctionType.Relu, bias=b1sb[:])
                    ps2 = mpp.tile([HID, 512], f32, tag="ps2")
                    nc.tensor.matmul(ps2[:, 0:cols], lhsT=W2sb[:], rhs=r1[:, 0:cols],
                                     start=True, stop=True)
                    hb = mpool.tile([HID, 512], bf16, tag="hb")
                    nc.scalar.activation(hb[:, 0:cols], ps2[:, 0:cols],
                                         mybir.ActivationFunctionType.Identity, bias=b2sb[:])
                    pst = mpp.tile([P, 4, HID], bf16, tag="pst")
                    for j in range(nbk):
                        nc.tensor.transpose(pst[:, j, :], hb[:, j * P:(j + 1) * P],
                                            identb[0:HID, 0:HID])
                    hts = hTD[:, 4 * it:4 * it + nbk, :]
                    nc.scalar.activation(hts, pst[:, 0:nbk, :],
                                         mybir.ActivationFunctionType.Copy)
                    sq = mpool.tile([P, 4, HID], bf16, tag="sq")
                    nc.vector.tensor_tensor(out=sq[:, 0:nbk, :], in0=hts, in1=hts,
                                            op=mybir.AluOpType.mult)
                    ss = mpool.tile([P, 4], f32, tag="ss")
                    nc.vector.tensor_reduce(out=ss[:, 0:nbk], in_=sq[:, 0:nbk, :],
                                            axis=mybir.AxisListType.X,
                                            op=mybir.AluOpType.add)
                    nrm = mpool.tile([P, 4], f32, tag="nrm")
                    nc.scalar.activation(nrm[:, 0:nbk], ss[:, 0:nbk],
                                         mybir.ActivationFunctionType.Sqrt)
                    nc.vector.tensor_scalar_max(out=nrm[:, 0:nbk], in0=nrm[:, 0:nbk],
                                                scalar1=1e-12)
                    rn = mpool.tile([P, 4], f32, tag="rn")
                    nc.vector.reciprocal(rn[:, 0:nbk], nrm[:, 0:nbk])
                    tabt = mpool.tile([P, 4, D2], bf16, tag="tabt")
                    nc.vector.tensor_tensor(
                        out=tabt[:, 0:nbk, 0:HID], in0=hts,
                        in1=rn[:, 0:nbk].unsqueeze(2).to_broadcast([P, nbk, HID]),
                        op=mybir.AluOpType.mult)
                    nc.vector.tensor_copy(tabt[:, 0:nbk, HID:D2], hts)
                    nc.sync.dma_start(out=agin[:, 4 * it:4 * it + nbk, :],
                                      in_=tabt[:, 0:nbk, :])
            # zero row for gather pads — AFTER the MLP stores (overwrites the
            # fake node at (p=127, b=97) = flat row 12543)
            nc.sync.dma_start(out=agin[P - 1:P, NB - 1:NB, :], in_=zrow[:])

            # self-loop partial (base order): exp(beta) * h
            with tc.tile_pool(name="selfp", bufs=1) as spool_:
                sagg = spool_.tile([P, NB, HID], f32)
                nc.vector.tensor_scalar_mul(out=sagg[:], in0=hTD[:], scalar1=sbeta[:])
                nc.sync.dma_start(out=sagg_t[:], in_=sagg[:])

            # ---------------- phase 2: AllGather the table -----------------
            nc.gpsimd.collective_compute(
                "AllGather",
                mybir.AluOpType.bypass,
                replica_groups=[list(range(CORES))],
                ins=[agin[:].flatten_outer_dims().opt()],
                outs=[table[:].opt()],
            )

            # ---------------- phase 2.5: per-chunk dst tables --------------
            with tc.tile_pool(name="hnprep", bufs=1) as hpool:
                aginflat = agin[:].flatten_outer_dims()
                for c in range(NCH):
                    sxi = hpool.tile([P, NPP // 16], mybir.dt.int16, tag="sxi")
                    nc.sync.dma_start(out=sxi[:], in_=sidx_t[c])
                    scr = hpool.tile([P, NB, D2], bf16, tag="scr")
                    nc.gpsimd.dma_gather(scr[:], aginflat, sxi[:], NPP, NPP, D2,
                                         single_packet=False)
                    nc.vector.tensor_copy(hnD[c][:], scr[:, :, 0:HID])

            # ---------------- phase 3: batched per-group AGNN partials -----
            # Stage-staggered: s1(dots+exp) for group g, s2(weights+PE) for
            # g-1, s3(psum copy+store) for g-2.
            with (
                tc.tile_pool(name="blk", bufs=3) as bpool,
                tc.tile_pool(name="blks", bufs=2) as spool,
                tc.tile_pool(name="sacc", bufs=2) as apool,
                tc.tile_pool(name="blkp", bufs=3, space="PSUM") as bpp,
            ):
                ngr = [len(groups[c]) for c in range(NCH)]
                done = [0] * NCH

                def s2(e):
                    c = e["c"]; nbk = e["nbk"]; Kg = e["Kg"]; nk = nbk * Kg
                    nc.vector.tensor_reduce(
                        out=stile[c][:, e["b0"]:e["b0"] + nbk], in_=e["ex2"][:],
                        axis=mybir.AxisListType.XY, op=mybir.AluOpType.add)
                    wts = spool.tile([P, nbk, Kg, HID], bf16, tag="wts")
                    nc.vector.tensor_tensor(
                        out=wts[:].rearrange("p n k (a b) -> p (n k) a b", b=2),
                        in0=e["slh"].rearrange("p c (a b) -> p c a b", b=2),
                        in1=e["ex2"][:].rearrange("p n k e -> p (n k) e")
                            .unsqueeze(2).to_broadcast([P, nk, 32, 2]),
                        op=mybir.AluOpType.mult)
                    pgrp = bpp.tile([P, MAXG, HID], f32, tag="pgrp")
                    for k in range(Kg):
                        nc.tensor.matmul(pgrp[:, 0:nbk, :], lhsT=identb[:],
                                         rhs=wts[:, :, k, :],
                                         start=(k == 0), stop=(k == Kg - 1))
                    e["pgrp"] = pgrp

                def s3(e):
                    c = e["c"]; nbk = e["nbk"]
                    aggsb = apool.tile([P, MAXG, HID], f32, tag="aggsb")
                    nc.scalar.activation(aggsb[:, 0:nbk, :], e["pgrp"][:, 0:nbk, :],
                                         mybir.ActivationFunctionType.Copy)
                    nc.sync.dma_start(out=agg_t[c, :, e["b0"]:e["b0"] + nbk, :],
                                      in_=aggsb[:, 0:nbk, :])
                    done[c] += 1
                    if done[c] == ngr[c]:
                        nc.sync.dma_start(out=s_t[c], in_=stile[c][:])

                prev1 = None
                prev2 = None
                with nc.allow_low_precision(reason="bf16 edge logits"):
                    for c in range(NCH):
                        for (col0, ncols, g0, ng) in batches[c]:
                            wc0, wcn = col0 * 8, ncols * 8
                            idxsb = bpool.tile([P, BATCH_COLS * 8],
                                               mybir.dt.int16, tag="idx")
                            nc.sync.dma_start(out=idxsb[:, 0:wcn],
                                              in_=idx_t[:, wc0:wc0 + wcn])
                            slots = bpool.tile([P, BATCH_COLS, D2], bf16,
                                               tag="slots")
                            nc.gpsimd.dma_gather(
                                slots[:, 0:ncols, :],
                                table[c * CHROWS:(c + 1) * CHROWS, :],
                                idxsb[:, 0:wcn],
                                ncols * P, ncols * P, D2, single_packet=False)
                            for gi in range(g0, g0 + ng):
                                (b0, nbk, Kg, gcol0) = groups[c][gi]
                                cb = gcol0 - col0
                                nk = nbk * Kg
                                slh = slots[:, cb:cb + nk, HID:D2]
                                sl4 = slots[:, cb:cb + nk, 0:HID].rearrange(
                                    "p (n k) e -> p n k e", k=Kg)
                                tmp = spool.tile([P, nbk, Kg, HID], bf16,
                                                 tag="tmp")
                                nc.vector.tensor_tensor(
                                    out=tmp[:], in0=sl4,
                                    in1=hnD[c][:, b0:b0 + nbk, :].unsqueeze(2)
                                        .to_broadcast([P, nbk, Kg, HID]),
                                    op=mybir.AluOpType.mult)
                                delta = spool.tile([P, nbk, Kg], bf16,
                                                   tag="delta")
                                nc.vector.tensor_reduce(
                                    out=delta[:], in_=tmp[:],
                                    axis=mybir.AxisListType.X,
                                    op=mybir.AluOpType.add)
                                ex2 = spool.tile([P, nbk, Kg, 2], bf16,
                                                 tag="ex2")
                                nc.scalar.activation(
                                    ex2[:],
                                    delta[:].unsqueeze(3)
                                        .to_broadcast([P, nbk, Kg, 2]),
                                    mybir.ActivationFunctionType.Exp,
                                    scale=betasb[:])
                                e = dict(c=c, b0=b0, nbk=nbk, Kg=Kg,
                                         ex2=ex2, slh=slh)
                                if prev1 is not None:
                                    s2(prev1)
                                if prev2 is not None:
                                    s3(prev2)
                                prev2, prev1 = prev1, e
                    if prev1 is not None:
                        s2(prev1)
                    if prev2 is not None:
                        s3(prev2)
                    if prev1 is not None:
                        s3(prev1)

        free_agin()
        free_table()

    nc.compile()
    _split_multi_waits(nc)
    return nc


# ------------------------------------------------------------------- driver
_CACHE = {}


def _get_nc(cfg_key, cfg):
    if cfg_key not in _CACHE:
        _CACHE[cfg_key] = _build_nc(cfg)
    return _CACHE[cfg_key]


def _make_in_maps(inputs, prep):
    x = np.asarray(inputs["x"], dtype=np.float32)
    W1 = np.asarray(inputs["W1"], dtype=np.float32)
    b1 = np.asarray(inputs["b1"], dtype=np.float32).reshape(-1, 1)
    W2 = np.asarray(inputs["W2"], dtype=np.float32)
    b2 = np.asarray(inputs["b2"], dtype=np.float32).reshape(-1, 1)
    beta = np.asarray(inputs["beta"], dtype=np.float32)
    beta128 = np.repeat(beta.reshape(1, 1), P, axis=0).astype(np.float32)
    import ml_dtypes
    bf = ml_dtypes.bfloat16

    in_maps = []
    for p in range(CORES):
        xp = x[p * NP:(p + 1) * NP]
        xpad = np.zeros((NPP, IN), np.float32)
        xpad[:NP] = xp
        in_maps.append({
            "xT": np.ascontiguousarray(xpad.T).astype(bf),
            "idx": prep["idx_all"][p],
            "sidx": prep["sidx_all"][p],
            "W1": W1.astype(bf), "b1": b1,
            "W2": W2.astype(bf), "b2": b2,
            "beta128": beta128,
        })
    return in_maps


def _postprocess(results, prep, inputs):
    Wc = np.asarray(inputs["Wc"], dtype=np.float64)
    bc = np.asarray(inputs["bc"], dtype=np.float64)
    beta = float(np.asarray(inputs["beta"]).reshape(-1)[0])
    expb = math.exp(beta)
    K = prep["Keff"]
    y = np.zeros((N, Wc.shape[1]), np.float32)
    n = np.arange(NP)
    bb, dd = n // P, n % P
    for p in range(CORES):
        res = results[p]
        agg = res["agg"].astype(np.float64)       # [4, 128, 98, 64]
        sdev = res["s"].astype(np.float64)        # [4, 128, 98]
        sagg = res["selfagg"].astype(np.float64)  # [128, 98, 64]
        acc = sagg[dd, bb, :].copy()              # self term, base order
        stot = np.full(NP, expb)
        cntp = prep["cnt"][p * NP:(p + 1) * NP]
        for c in range(NCH):
            pos = prep["pos_pc"][p, c]
            b_, d_ = pos // P, pos % P
            acc += agg[c, d_, b_, :]
            stot += sdev[c, d_, b_] / 2.0 - (K[c, b_] - cntp[:, c])
        y[p * NP:(p + 1) * NP] = ((acc / stot[:, None]) @ Wc + bc).astype(np.float32)
    return y


def get_nc_for_test(inputs):
    prep = _preprocess(np.asarray(inputs["edge_index"]))
    cfg = dict(K=prep["K"], TOTC=prep["TOTC"], groups=prep["groups"], batches=prep["batches"])
    cfg_key = ("v6", tuple(int(k) for k in prep["Keff"].ravel()))
    return _get_nc(cfg_key, cfg)


def kernel(**inputs):
    from concourse.bass_utils import run_bass_kernel_spmd

    prep = _preprocess(np.asarray(inputs["edge_index"]))
    cfg = dict(K=prep["K"], TOTC=prep["TOTC"], groups=prep["groups"], batches=prep["batches"])
    cfg_key = ("v6", tuple(int(k) for k in prep["Keff"].ravel()))
    nc = _get_nc(cfg_key, cfg)
    in_maps = _make_in_maps(inputs, prep)
    res = run_bass_kernel_spmd(nc, in_maps, core_ids=list(range(CORES)))
    return _postprocess(res.results, prep, inputs)


if __name__ == "__main__":
    pass



# revision 5
# speedup vs baseline: 1.0428x; 1.0413x over previous
"""AGNN (AMNet) message-passing kernel for 8 TRN2 NeuronCores.

Design (vs the v1 baseline):
  - Nodes partitioned contiguously across 8 cores (no global sort).
  - Phase 1: input MLP in bf16, 4 blocks (512 nodes) per iteration; packed
    [hn|h] bf16 rows stored to a local HBM table `agin` in partition-major
    layout (node n -> flat row (n%128)*98 + n//128).
  - Phase 2: ONE AllGather replicates the table (addr_space Shared).
  - Per-chunk independent dst orderings: for each of the 4 table chunks
    (int16 gather reach), each core re-sorts its OWN dsts by that chunk's
    in-degree, making the per-(block,chunk) rectangle schedule tight
    (~1.1x padding vs 2.07x for a common ordering).
  - Device outputs per-chunk partial aggregates (fp32) and exp-sums; the
    softmax combine across chunks + self-loop term + classifier run on the
    HOST (they commute with the chunk decomposition).
  - No tile_critical / manual semaphores: tile auto-tracks dma_gather.

kernel() accepts FULL inputs and returns the FULL [N, 2] float32 output.
"""

import math
import os
import sys

sys.path.insert(0, "/opt/trn_rl_repo")

import numpy as np

CORES = 8
P = 128
NCH = 4
N = 100000
NP = N // CORES            # 12500
NB = math.ceil(NP / P)     # 98
NPP = NB * P               # 12544
CHROWS = 2 * NPP           # 25088
ZROW = NPP - 1             # 12543  == (12543%128)*98 + 12543//128
SBB = 8                    # blocks per super-block (one gather each)
GB = SBB                   # psum->sbuf copy group
IN, HID = 128, 64
D2 = 2 * HID
NIT = NB // 4 + (1 if NB % 4 else 0)  # 25 MLP iterations (4 blocks each)

AGG_DVE = bool(os.environ.get("AGG_DVE"))  # A/B: aggregation on DVE vs PE
MAXG = 8                       # max blocks per compute group (PSUM)
MAXGC = 48                     # max columns per compute group (SBUF)
BATCH_COLS = 64                # gather batch size (columns)
CCOL = 330.0                   # DP cost per padded column (ns-ish)
CGRP = 1500.0                  # DP cost per extra group


def _dp_groups(Kc):
    """Consecutive groups (<=MAXG blocks, <=MAXGC cols) minimizing
    sum(len*Kmax*CCOL + CGRP)."""
    nb = len(Kc)
    best = [float("inf")] * (nb + 1)
    prev = [0] * (nb + 1)
    best[0] = 0.0
    for i in range(1, nb + 1):
        kmax = 0
        for L in range(1, MAXG + 1):
            j = i - L
            if j < 0:
                break
            kmax = max(kmax, Kc[j])
            if L * kmax > MAXGC:
                break
            cst = best[j] + L * kmax * CCOL + CGRP
            if cst < best[i]:
                best[i] = cst
                prev[i] = j
    out = []
    i = nb
    while i > 0:
        j = prev[i]
        out.append((j, i - j, int(max(Kc[j:i]))))
        i = j
    return out[::-1]


def _flatrow(n):
    """Local table row of local node id n (partition-major layout)."""
    return (n % P) * NB + n // P


def _wrap_idx(flat):
    n = flat.shape[0]
    w = flat.reshape(n // 16, 16).T
    return np.tile(w, (8, 1)).astype(np.int16)


# ----------------------------------------------------------------- host prep
def _preprocess(edge_index):
    src = np.asarray(edge_index[0], dtype=np.int64)
    dst = np.asarray(edge_index[1], dtype=np.int64)
    chunk_of_src = src // (2 * NP)

    # CSR sorted by (dst, chunk)
    order = np.lexsort((src, chunk_of_src, dst))
    src_s = src[order]
    ch_s = chunk_of_src[order]
    # per (dst, chunk) counts
    cnt = np.zeros((N, NCH), np.int64)
    np.add.at(cnt, (dst, chunk_of_src), 1)
    cnt_cum = np.concatenate(
        [np.zeros((N, 1), np.int64), np.cumsum(cnt, axis=1)], axis=1
    )  # [N, 5]
    deg = cnt.sum(axis=1)
    row_start = np.zeros(N + 1, np.int64)
    np.cumsum(deg, out=row_start[1:])

    # local table row of each src (within its chunk)
    q = src_s // NP
    nloc = src_s % NP
    locrow = (q % 2) * NPP + (nloc % P) * NB + nloc // P  # within-chunk row

    # per-(core, chunk) orderings + per-block maxima
    pos_pc = np.zeros((CORES, NCH, NP), np.int64)   # node -> position
    Kpc = np.zeros((CORES, NCH, NB), np.int64)
    for p in range(CORES):
        lo = p * NP
        cl = cnt[lo:lo + NP]                         # [NP, NCH]
        for c in range(NCH):
            o = np.argsort(-cl[:, c], kind="stable")
            pos = np.empty(NP, np.int64)
            pos[o] = np.arange(NP)
            pos_pc[p, c] = pos
            srt = cl[o, c]
            padded = np.zeros(NPP, np.int64)
            padded[:NP] = srt
            Kpc[p, c] = padded.reshape(NB, P).max(axis=1)
    K = Kpc.max(axis=0)                              # [NCH, NB]
    K = np.maximum(K, 1)

    # flexible uniform-K groups per chunk + gather batches of whole groups
    colstart = np.zeros((NCH, NB), np.int64)
    Keff = np.zeros((NCH, NB), np.int64)
    groups = []   # [NCH] list of (b0, nbk, Kg, col0)  (col0 global)
    batches = []  # [NCH] list of (col0, ncols, g0, ng)
    coff = 0
    for c in range(NCH):
        gs = _dp_groups(list(K[c]))
        glist = []
        bl = []
        bc0, bg0, bnc = coff, 0, 0
        for gi, (b0, nbk, Kg) in enumerate(gs):
            w = nbk * Kg
            if bnc + w > BATCH_COLS and bnc > 0:
                bl.append((bc0, bnc, bg0, gi - bg0))
                bc0, bg0, bnc = coff, gi, 0
            glist.append((b0, nbk, Kg, coff))
            Keff[c, b0:b0 + nbk] = Kg
            for j in range(nbk):
                colstart[c, b0 + j] = coff + j * Kg
            coff += w
            bnc += w
        bl.append((bc0, bnc, bg0, len(gs) - bg0))
        groups.append(glist)
        batches.append(bl)
    TOTC = coff

    # index streams
    idx_all = np.zeros((CORES, P, TOTC * 8), np.int16)
    sidx_all = np.zeros((CORES, NCH, P, NPP // 16), np.int16)
    for p in range(CORES):
        lo = p * NP
        e0, e1 = row_start[lo], row_start[lo + NP]
        d_e = dst[order[e0:e1]] - lo
        c_e = ch_s[e0:e1]
        r_e = locrow[e0:e1]
        # k-rank of each edge within its (dst, chunk) segment
        seg0 = row_start[d_e + lo] - e0 + cnt_cum[d_e + lo, c_e]
        k_e = np.arange(e1 - e0) - seg0
        A = np.full((TOTC, P), ZROW, np.int16)
        pos_e = pos_pc[p, c_e, d_e]
        col_e = colstart[c_e, pos_e // P] + k_e
        A[col_e, pos_e % P] = r_e.astype(np.int16)
        idx_all[p] = _wrap_idx(A.ravel())
        for c in range(NCH):
            o = np.argsort(pos_pc[p, c], kind="stable")  # position -> node
            rows = np.full(NPP, ZROW, np.int64)
            rows[:NP] = _flatrow(o)
            sidx_all[p, c] = _wrap_idx(rows)

    return dict(K=K, Keff=Keff, TOTC=TOTC, groups=groups, batches=batches,
                colstart=colstart,
                pos_pc=pos_pc, cnt=cnt, idx_all=idx_all, sidx_all=sidx_all)


# ------------------------------------------------------------------ builder
def _patch_walrus_args():
    import concourse.bass_utils as bu
    if getattr(bu, "_agnn_dge_patch", False):
        return
    orig = bu.get_walrus_args

    def patched(*a, **k):
        return list(orig(*a, **k)) + [
            "--dge-levels=io,spill_reload,scalar_dynamic_offset,"
            "vector_dynamic_offsets,dst_reduce,transpose",
        ]

    bu.get_walrus_args = patched
    bu._agnn_dge_patch = True


def _split_multi_waits(nc):
    import bass_rust
    import concourse.mybir as mybir

    cnt = 0
    for func in nc.m.functions:
        for block in func.blocks:
            out = []
            for inst in block.instructions:
                si = inst.sync_info
                cap = 2 if isinstance(inst, mybir.InstEventSemaphore) else 1
                if (si is not None and si.on_wait and len(si.on_wait) > cap
                        and inst.engine is not None):
                    waits = list(si.on_wait)
                    for w in waits[cap:]:
                        cnt += 1
                        nop = mybir.InstNoOp(
                            name=f"wsplit{cnt}", engine=inst.engine, ins=[], outs=[]
                        )
                        nop.sync_info = bass_rust.SyncInfo(on_wait=[w], on_update=[])
                        try:
                            nc.register_instruction(nop, overwrite=True)
                        except Exception:
                            pass
                        out.append(nop)
                    inst.sync_info = bass_rust.SyncInfo(
                        on_wait=waits[:cap], on_update=list(si.on_update or [])
                    )
                out.append(inst)
            try:
                block.instructions = out
            except Exception:
                block.instructions.clear()
                block.instructions.extend(out)
    return cnt


def _build_nc(cfg):
    _patch_walrus_args()
    import concourse.bacc as bacc
    import concourse.mybir as mybir
    import concourse.tile as tile
    from concourse.masks import make_identity

    K = cfg["K"]; TOTC = cfg["TOTC"]
    groups = cfg["groups"]; batches = cfg["batches"]

    f32 = mybir.dt.float32
    bf16 = mybir.dt.bfloat16

    nc = bacc.Bacc("TRN2", num_devices=CORES, dynamic_dma_scratch_size=65536)

    xT_t = nc.declare_dram_parameter("xT", [IN, NPP], bf16, isOutput=False)
    idx_t = nc.declare_dram_parameter("idx", [P, TOTC * 8], mybir.dt.int16, isOutput=False)
    sidx_t = nc.declare_dram_parameter("sidx", [NCH, P, NPP // 16], mybir.dt.int16, isOutput=False)
    W1_t = nc.declare_dram_parameter("W1", [IN, HID], bf16, isOutput=False)
    b1_t = nc.declare_dram_parameter("b1", [HID, 1], f32, isOutput=False)
    W2_t = nc.declare_dram_parameter("W2", [HID, HID], bf16, isOutput=False)
    b2_t = nc.declare_dram_parameter("b2", [HID, 1], f32, isOutput=False)
    beta_t = nc.declare_dram_parameter("beta128", [P, 1], f32, isOutput=False)
    agg_t = nc.declare_dram_parameter("agg", [NCH, P, NB, HID], bf16, isOutput=True)
    s_t = nc.declare_dram_parameter("s", [NCH, P, NB], f32, isOutput=True)
    sagg_t = nc.declare_dram_parameter("selfagg", [P, NB, HID], f32, isOutput=True)

    with tile.TileContext(nc) as tc:
        agin, free_agin = tc.tile([P, NB, D2], bf16, space="DRAM", name="agin")
        import contextlib
        _tabctx = contextlib.ExitStack()
        _tabpool = _tabctx.enter_context(
            tc.tile_pool(name="tabpool", space="DRAM", bufs=1))
        table = _tabpool.tile([CORES * NPP, D2], bf16, name="table",
                              addr_space="Shared")

        def free_table():
            _tabctx.close()

        with tc.tile_pool(name="consts", bufs=1) as cpool:
            W1sb = cpool.tile([IN, HID], bf16)
            W2sb = cpool.tile([HID, HID], bf16)
            b1sb = cpool.tile([HID, 1], f32)
            b2sb = cpool.tile([HID, 1], f32)
            betasb = cpool.tile([P, 1], f32)
            sbeta = cpool.tile([P, 1], f32)
            ident = cpool.tile([P, P], f32)
            identb = cpool.tile([P, P], bf16)
            hTD = cpool.tile([P, NB, HID], bf16)
            zrow = cpool.tile([1, 1, D2], bf16)
            hnD = [cpool.tile([P, NB, HID], bf16, name=f"hnD{c}") for c in range(NCH)]

            nc.sync.dma_start(out=W1sb[:], in_=W1_t[:])
            nc.sync.dma_start(out=W2sb[:], in_=W2_t[:])
            nc.sync.dma_start(out=b1sb[:], in_=b1_t[:])
            nc.sync.dma_start(out=b2sb[:], in_=b2_t[:])
            nc.sync.dma_start(out=betasb[:], in_=beta_t[:])
            make_identity(nc, ident[:])
            nc.vector.tensor_copy(identb[:], ident[:])
            nc.scalar.activation(sbeta[:], betasb[:], mybir.ActivationFunctionType.Exp)
            nc.gpsimd.memset(zrow[:], 0)
            stile = [cpool.tile([P, NB], f32, name=f"stile{c}") for c in range(NCH)]

            # ---------------- phase 1: MLP + packed local table ------------
            with (
                tc.tile_pool(name="mlp", bufs=3) as mpool,
                tc.tile_pool(name="mlpp", bufs=2, space="PSUM") as mpp,
            ):
                for it in range(NIT):
                    nbk = min(4, NB - 4 * it)
                    cols = nbk * P
                    s0 = it * 4 * P
                    xt = mpool.tile([IN, 512], bf16, tag="xt")
                    nc.sync.dma_start(out=xt[:, 0:cols], in_=xT_t[:, s0:s0 + cols])
                    ps1 = mpp.tile([HID, 512], f32, tag="ps1")
                    nc.tensor.matmul(ps1[:, 0:cols], lhsT=W1sb[:], rhs=xt[:, 0:cols],
                                     start=True, stop=True)
                    r1 = mpool.tile([HID, 512], bf16, tag="r1")
                    nc.scalar.activation(r1[:, 0:cols], ps1[:, 0:cols],
                                         mybir.ActivationFunctionType.Relu, bias=b1sb[:])
                    ps2 = mpp.tile([HID, 512], f32, tag="ps2")
                    nc.tensor.matmul(ps2[:, 0:cols], lhsT=W2sb[:], rhs=r1[:, 0:cols],
                                     start=True, stop=True)
                    hb = mpool.tile([HID, 512], bf16, tag="hb")
                    nc.scalar.activation(hb[:, 0:cols], ps2[:, 0:cols],
                                         mybir.ActivationFunctionType.Identity, bias=b2sb[:])
                    pst = mpp.tile([P, 4, HID], bf16, tag="pst")
                    for j in range(nbk):
                        nc.tensor.transpose(pst[:, j, :], hb[:, j * P:(j + 1) * P],
                                            identb[0:HID, 0:HID])
                    hts = hTD[:, 4 * it:4 * it + nbk, :]
                    nc.scalar.activation(hts, pst[:, 0:nbk, :],
                                         mybir.ActivationFunctionType.Copy)
                    sq = mpool.tile([P, 4, HID], bf16, tag="sq")
                    nc.vector.tensor_tensor(out=sq[:, 0:nbk, :], in0=hts, in1=hts,
                                            op=mybir.AluOpType.mult)
                    ss = mpool.tile([P, 4], f32, tag="ss")
                    nc.vector.tensor_reduce(out=ss[:, 0:nbk], in_=sq[:, 0:nbk, :],
                                            axis=mybir.AxisListType.X,
                                            op=mybir.AluOpType.add)
                    nrm = mpool.tile([P, 4], f32, tag="nrm")
                    nc.scalar.activation(nrm[:, 0:nbk], ss[:, 0:nbk],
                                         mybir.ActivationFunctionType.Sqrt)
                    nc.vector.tensor_scalar_max(out=nrm[:, 0:nbk], in0=nrm[:, 0:nbk],
                                                scalar1=1e-12)
                    rn = mpool.tile([P, 4], f32, tag="rn")
                    nc.vector.reciprocal(rn[:, 0:nbk], nrm[:, 0:nbk])
                    tabt = mpool.tile([P, 4, D2], bf16, tag="tabt")
                    nc.vector.tensor_tensor(
                        out=tabt[:, 0:nbk, 0:HID], in0=hts,
                        in1=rn[:, 0:nbk].unsqueeze(2).to_broadcast([P, nbk, HID]),
                        op=mybir.AluOpType.mult)
                    nc.vector.tensor_copy(tabt[:, 0:nbk, HID:D2], hts)
                    nc.sync.dma_start(out=agin[:, 4 * it:4 * it + nbk, :],
                                      in_=tabt[:, 0:nbk, :])
            # zero row for gather pads — AFTER the MLP stores (overwrites the
            # fake node at (p=127, b=97) = flat row 12543)
            nc.sync.dma_start(out=agin[P - 1:P, NB - 1:NB, :], in_=zrow[:])

            # self-loop partial (base order): exp(beta) * h
            with tc.tile_pool(name="selfp", bufs=1) as spool_:
                sagg = spool_.tile([P, NB, HID], f32)
                nc.vector.tensor_scalar_mul(out=sagg[:], in0=hTD[:], scalar1=sbeta[:])
                nc.sync.dma_start(out=sagg_t[:], in_=sagg[:])

            # ---------------- phase 2: AllGather the table -----------------
            nc.gpsimd.collective_compute(
                "AllGather",
                mybir.AluOpType.bypass,
                replica_groups=[list(range(CORES))],
                ins=[agin[:].flatten_outer_dims().opt()],
                outs=[table[:].opt()],
            )

            # ---------------- phase 2.5: per-chunk dst tables --------------
            with tc.tile_pool(name="hnprep", bufs=1) as hpool:
                aginflat = agin[:].flatten_outer_dims()
                for c in range(NCH):
                    sxi = hpool.tile([P, NPP // 16], mybir.dt.int16, tag="sxi")
                    nc.sync.dma_start(out=sxi[:], in_=sidx_t[c])
                    scr = hpool.tile([P, NB, D2], bf16, tag="scr")
                    nc.gpsimd.dma_gather(scr[:], aginflat, sxi[:], NPP, NPP, D2,
                                         single_packet=False)
                    nc.vector.tensor_copy(hnD[c][:], scr[:, :, 0:HID])

            # ---------------- phase 3: batched per-group AGNN partials -----
            # Stage-staggered: s1(dots+exp) for group g, s2(weights+PE) for
            # g-1, s3(psum copy+store) for g-2.
            with (
                tc.tile_pool(name="blk", bufs=4) as bpool,
                tc.tile_pool(name="blks", bufs=2) as spool,
                tc.tile_pool(name="sacc", bufs=2) as apool,
                tc.tile_pool(name="blkp", bufs=3, space="PSUM") as bpp,
            ):
                ngr = [len(groups[c]) for c in range(NCH)]
                done = [0] * NCH

                def s2(e):
                    c = e["c"]; nbk = e["nbk"]; Kg = e["Kg"]; nk = nbk * Kg
                    nc.vector.tensor_reduce(
                        out=stile[c][:, e["b0"]:e["b0"] + nbk], in_=e["ex2"][:],
                        axis=mybir.AxisListType.XY, op=mybir.AluOpType.add)
                    wts = spool.tile([P, nbk, Kg, HID], bf16, tag="wts")
                    nc.vector.tensor_tensor(
                        out=wts[:].rearrange("p n k (a b) -> p (n k) a b", b=2),
                        in0=e["slh"].rearrange("p c (a b) -> p c a b", b=2),
                        in1=e["ex2"][:].rearrange("p n k e -> p (n k) e")
                            .unsqueeze(2).to_broadcast([P, nk, 32, 2]),
                        op=mybir.AluOpType.mult)
                    pgrp = bpp.tile([P, MAXG, HID], f32, tag="pgrp")
                    for k in range(Kg):
                        nc.tensor.matmul(pgrp[:, 0:nbk, :], lhsT=identb[:],
                                         rhs=wts[:, :, k, :],
                                         start=(k == 0), stop=(k == Kg - 1))
                    e["pgrp"] = pgrp

                def s3(e):
                    c = e["c"]; nbk = e["nbk"]
                    aggsb = apool.tile([P, MAXG, HID], bf16, tag="aggsb")
                    nc.scalar.activation(aggsb[:, 0:nbk, :], e["pgrp"][:, 0:nbk, :],
                                         mybir.ActivationFunctionType.Copy)
                    nc.sync.dma_start(out=agg_t[c, :, e["b0"]:e["b0"] + nbk, :],
                                      in_=aggsb[:, 0:nbk, :])
                    done[c] += 1
                    if done[c] == ngr[c]:
                        nc.sync.dma_start(out=s_t[c], in_=stile[c][:])

                prev1 = None
                prev2 = None
                with nc.allow_low_precision(reason="bf16 edge logits"):
                    for c in range(NCH):
                        for (col0, ncols, g0, ng) in batches[c]:
                            wc0, wcn = col0 * 8, ncols * 8
                            idxsb = bpool.tile([P, BATCH_COLS * 8],
                                               mybir.dt.int16, tag="idx")
                            nc.sync.dma_start(out=idxsb[:, 0:wcn],
                                              in_=idx_t[:, wc0:wc0 + wcn])
                            slots = bpool.tile([P, BATCH_COLS, D2], bf16,
                                               tag="slots")
                            nc.gpsimd.dma_gather(
                                slots[:, 0:ncols, :],
                                table[c * CHROWS:(c + 1) * CHROWS, :],
                                idxsb[:, 0:wcn],
                                ncols * P, ncols * P, D2, single_packet=False)
                            for gi in range(g0, g0 + ng):
                                (b0, nbk, Kg, gcol0) = groups[c][gi]
                                cb = gcol0 - col0
                                nk = nbk * Kg
                                slh = slots[:, cb:cb + nk, HID:D2]
                                sl4 = slots[:, cb:cb + nk, 0:HID].rearrange(
                                    "p (n k) e -> p n k e", k=Kg)
                                tmp = spool.tile([P, nbk, Kg, HID], bf16,
                                                 tag="tmp")
                                nc.vector.tensor_tensor(
                                    out=tmp[:], in0=sl4,
                                    in1=hnD[c][:, b0:b0 + nbk, :].unsqueeze(2)
                                        .to_broadcast([P, nbk, Kg, HID]),
                                    op=mybir.AluOpType.mult)
                                delta = spool.tile([P, nbk, Kg], bf16,
                                                   tag="delta")
                                nc.vector.tensor_reduce(
                                    out=delta[:], in_=tmp[:],
                                    axis=mybir.AxisListType.X,
                                    op=mybir.AluOpType.add)
                                ex2 = spool.tile([P, nbk, Kg, 2], bf16,
                                                 tag="ex2")
                                nc.scalar.activation(
                                    ex2[:],
                                    delta[:].unsqueeze(3)
                                        .to_broadcast([P, nbk, Kg, 2]),
                                    mybir.ActivationFunctionType.Exp,
                                    scale=betasb[:])
                                e = dict(c=c, b0=b0, nbk=nbk, Kg=Kg,
                                         ex2=ex2, slh=slh)
                                if prev1 is not None:
                                    s2(prev1)
                                if prev2 is not None:
                                    s3(prev2)
                                prev2, prev1 = prev1, e
                    if prev1 is not None:
                        s2(prev1)
                    if prev2 is not None:
                        s3(prev2)
                    if prev1 is not None:
                        s3(prev1)

        free_agin()
        free_table()

    nc.compile()
    _split_multi_waits(nc)
    return nc


# ------------------------------------------------------------------- driver
_CACHE = {}


def _get_nc(cfg_key, cfg):
    if cfg_key not in _CACHE:
        _CACHE[cfg_key] = _build_nc(cfg)
    return _CACHE[cfg_key]


def _make_in_maps(inputs, prep):
    x = np.asarray(inputs["x"], dtype=np.float32)
    W1 = np.asarray(inputs["W1"], dtype=np.float32)
    b1 = np.asarray(inputs["b1"], dtype=np.float32).reshape(-1, 1)
    W2 = np.asarray(inputs["W2"], dtype=np.float32)
    b2 = np.asarray(inputs["b2"], dtype=np.float32).reshape(-1, 1)
    beta = np.asarray(inputs["beta"], dtype=np.float32)
    beta128 = np.repeat(beta.reshape(1, 1), P, axis=0).astype(np.float32)
    import ml_dtypes
    bf = ml_dtypes.bfloat16

    in_maps = []
    for p in range(CORES):
        xp = x[p * NP:(p + 1) * NP]
        xpad = np.zeros((NPP, IN), np.float32)
        xpad[:NP] = xp
        in_maps.append({
            "xT": np.ascontiguousarray(xpad.T).astype(bf),
            "idx": prep["idx_all"][p],
            "sidx": prep["sidx_all"][p],
            "W1": W1.astype(bf), "b1": b1,
            "W2": W2.astype(bf), "b2": b2,
            "beta128": beta128,
        })
    return in_maps


def _postprocess(results, prep, inputs):
    Wc = np.asarray(inputs["Wc"], dtype=np.float64)
    bc = np.asarray(inputs["bc"], dtype=np.float64)
    beta = float(np.asarray(inputs["beta"]).reshape(-1)[0])
    expb = math.exp(beta)
    K = prep["Keff"]
    y = np.zeros((N, Wc.shape[1]), np.float32)
    n = np.arange(NP)
    bb, dd = n // P, n % P
    for p in range(CORES):
        res = results[p]
        agg = res["agg"].astype(np.float64)       # [4, 128, 98, 64]
        sdev = res["s"].astype(np.float64)        # [4, 128, 98]
        sagg = res["selfagg"].astype(np.float64)  # [128, 98, 64]
        acc = sagg[dd, bb, :].copy()              # self term, base order
        stot = np.full(NP, expb)
        cntp = prep["cnt"][p * NP:(p + 1) * NP]
        for c in range(NCH):
            pos = prep["pos_pc"][p, c]
            b_, d_ = pos // P, pos % P
            acc += agg[c, d_, b_, :]
            stot += sdev[c, d_, b_] / 2.0 - (K[c, b_] - cntp[:, c])
        y[p * NP:(p + 1) * NP] = ((acc / stot[:, None]) @ Wc + bc).astype(np.float32)
    return y


def get_nc_for_test(inputs):
    prep = _preprocess(np.asarray(inputs["edge_index"]))
    cfg = dict(K=prep["K"], TOTC=prep["TOTC"], groups=prep["groups"], batches=prep["batches"])
    cfg_key = ("v6", tuple(int(k) for k in prep["Keff"].ravel()))
    return _get_nc(cfg_key, cfg)


def kernel(**inputs):
    from concourse.bass_utils import run_bass_kernel_spmd

    prep = _preprocess(np.asarray(inputs["edge_index"]))
    cfg = dict(K=prep["K"], TOTC=prep["TOTC"], groups=prep["groups"], batches=prep["batches"])
    cfg_key = ("v6", tuple(int(k) for k in prep["Keff"].ravel()))
    nc = _get_nc(cfg_key, cfg)
    in_maps = _make_in_maps(inputs, prep)
    res = run_bass_kernel_spmd(nc, in_maps, core_ids=list(range(CORES)))
    return _postprocess(res.results, prep, inputs)


if __name__ == "__main__":
    pass



# revision 6
# speedup vs baseline: 1.0790x; 1.0347x over previous
"""AGNN (AMNet) message-passing kernel for 8 TRN2 NeuronCores.

Design (vs the v1 baseline):
  - Nodes partitioned contiguously across 8 cores (no global sort).
  - Phase 1: input MLP in bf16, 4 blocks (512 nodes) per iteration; packed
    [hn|h] bf16 rows stored to a local HBM table `agin` in partition-major
    layout (node n -> flat row (n%128)*98 + n//128).
  - Phase 2: ONE AllGather replicates the table (addr_space Shared).
  - Per-chunk independent dst orderings: for each of the 4 table chunks
    (int16 gather reach), each core re-sorts its OWN dsts by that chunk's
    in-degree, making the per-(block,chunk) rectangle schedule tight
    (~1.1x padding vs 2.07x for a common ordering).
  - Device outputs per-chunk partial aggregates (fp32) and exp-sums; the
    softmax combine across chunks + self-loop term + classifier run on the
    HOST (they commute with the chunk decomposition).
  - No tile_critical / manual semaphores: tile auto-tracks dma_gather.

kernel() accepts FULL inputs and returns the FULL [N, 2] float32 output.
"""

import math
import os
import sys

sys.path.insert(0, "/opt/trn_rl_repo")

import numpy as np

CORES = 8
P = 128
NCH = 4
N = 100000
NP = N // CORES            # 12500
NB = math.ceil(NP / P)     # 98
NPP = NB * P               # 12544
CHROWS = 2 * NPP           # 25088
ZROW = NPP - 1             # 12543  == (12543%128)*98 + 12543//128
SBB = 8                    # blocks per super-block (one gather each)
GB = SBB                   # psum->sbuf copy group
IN, HID = 128, 64
D2 = 2 * HID
NIT = NB // 4 + (1 if NB % 4 else 0)  # 25 MLP iterations (4 blocks each)

AGG_DVE = bool(os.environ.get("AGG_DVE"))  # A/B: aggregation on DVE vs PE
MAXG = 8                       # max blocks per compute group (PSUM)
MAXGC = 48                     # max columns per compute group (SBUF)
BATCH_COLS = 48                # gather batch size (columns)
CCOL = 330.0                   # DP cost per padded column (ns-ish)
CGRP = 1500.0                  # DP cost per extra group


def _dp_groups(Kc):
    """Consecutive groups (<=MAXG blocks, <=MAXGC cols) minimizing
    sum(len*Kmax*CCOL + CGRP)."""
    nb = len(Kc)
    best = [float("inf")] * (nb + 1)
    prev = [0] * (nb + 1)
    best[0] = 0.0
    for i in range(1, nb + 1):
        kmax = 0
        for L in range(1, MAXG + 1):
            j = i - L
            if j < 0:
                break
            kmax = max(kmax, Kc[j])
            if L * kmax > MAXGC:
                break
            cst = best[j] + L * kmax * CCOL + CGRP
            if cst < best[i]:
                best[i] = cst
                prev[i] = j
    out = []
    i = nb
    while i > 0:
        j = prev[i]
        out.append((j, i - j, int(max(Kc[j:i]))))
        i = j
    return out[::-1]


def _flatrow(n):
    """Local table row of local node id n (partition-major layout)."""
    return (n % P) * NB + n // P


def _wrap_idx(flat):
    n = flat.shape[0]
    w = flat.reshape(n // 16, 16).T
    return np.tile(w, (8, 1)).astype(np.int16)


# ----------------------------------------------------------------- host prep
def _preprocess(edge_index):
    src = np.asarray(edge_index[0], dtype=np.int64)
    dst = np.asarray(edge_index[1], dtype=np.int64)
    chunk_of_src = src // (2 * NP)

    # CSR sorted by (dst, chunk)
    order = np.lexsort((src, chunk_of_src, dst))
    src_s = src[order]
    ch_s = chunk_of_src[order]
    # per (dst, chunk) counts
    cnt = np.zeros((N, NCH), np.int64)
    np.add.at(cnt, (dst, chunk_of_src), 1)
    cnt_cum = np.concatenate(
        [np.zeros((N, 1), np.int64), np.cumsum(cnt, axis=1)], axis=1
    )  # [N, 5]
    deg = cnt.sum(axis=1)
    row_start = np.zeros(N + 1, np.int64)
    np.cumsum(deg, out=row_start[1:])

    # local table row of each src (within its chunk)
    q = src_s // NP
    nloc = src_s % NP
    locrow = (q % 2) * NPP + (nloc % P) * NB + nloc // P  # within-chunk row

    # per-(core, chunk) orderings + per-block maxima
    pos_pc = np.zeros((CORES, NCH, NP), np.int64)   # node -> position
    Kpc = np.zeros((CORES, NCH, NB), np.int64)
    for p in range(CORES):
        lo = p * NP
        cl = cnt[lo:lo + NP]                         # [NP, NCH]
        for c in range(NCH):
            o = np.argsort(-cl[:, c], kind="stable")
            pos = np.empty(NP, np.int64)
            pos[o] = np.arange(NP)
            pos_pc[p, c] = pos
            srt = cl[o, c]
            padded = np.zeros(NPP, np.int64)
            padded[:NP] = srt
            Kpc[p, c] = padded.reshape(NB, P).max(axis=1)
    K = Kpc.max(axis=0)                              # [NCH, NB]
    K = np.maximum(K, 1)

    # flexible uniform-K groups per chunk + gather batches of whole groups
    colstart = np.zeros((NCH, NB), np.int64)
    Keff = np.zeros((NCH, NB), np.int64)
    groups = []   # [NCH] list of (b0, nbk, Kg, col0)  (col0 global)
    batches = []  # [NCH] list of (col0, ncols, g0, ng)
    coff = 0
    for c in range(NCH):
        gs = _dp_groups(list(K[c]))
        glist = []
        bl = []
        bc0, bg0, bnc = coff, 0, 0
        for gi, (b0, nbk, Kg) in enumerate(gs):
            w = nbk * Kg
            if bnc + w > BATCH_COLS and bnc > 0:
                bl.append((bc0, bnc, bg0, gi - bg0))
                bc0, bg0, bnc = coff, gi, 0
            glist.append((b0, nbk, Kg, coff))
            Keff[c, b0:b0 + nbk] = Kg
            for j in range(nbk):
                colstart[c, b0 + j] = coff + j * Kg
            coff += w
            bnc += w
        bl.append((bc0, bnc, bg0, len(gs) - bg0))
        groups.append(glist)
        batches.append(bl)
    TOTC = coff

    # index streams
    idx_all = np.zeros((CORES, P, TOTC * 8), np.int16)
    sidx_all = np.zeros((CORES, NCH, P, NPP // 16), np.int16)
    for p in range(CORES):
        lo = p * NP
        e0, e1 = row_start[lo], row_start[lo + NP]
        d_e = dst[order[e0:e1]] - lo
        c_e = ch_s[e0:e1]
        r_e = locrow[e0:e1]
        # k-rank of each edge within its (dst, chunk) segment
        seg0 = row_start[d_e + lo] - e0 + cnt_cum[d_e + lo, c_e]
        k_e = np.arange(e1 - e0) - seg0
        A = np.full((TOTC, P), ZROW, np.int16)
        pos_e = pos_pc[p, c_e, d_e]
        col_e = colstart[c_e, pos_e // P] + k_e
        A[col_e, pos_e % P] = r_e.astype(np.int16)
        idx_all[p] = _wrap_idx(A.ravel())
        for c in range(NCH):
            o = np.argsort(pos_pc[p, c], kind="stable")  # position -> node
            rows = np.full(NPP, ZROW, np.int64)
            rows[:NP] = _flatrow(o)
            sidx_all[p, c] = _wrap_idx(rows)

    return dict(K=K, Keff=Keff, TOTC=TOTC, groups=groups, batches=batches,
                colstart=colstart,
                pos_pc=pos_pc, cnt=cnt, idx_all=idx_all, sidx_all=sidx_all)


# ------------------------------------------------------------------ builder
def _patch_walrus_args():
    import concourse.bass_utils as bu
    if getattr(bu, "_agnn_dge_patch", False):
        return
    orig = bu.get_walrus_args

    def patched(*a, **k):
        return list(orig(*a, **k)) + [
            "--dge-levels=io,spill_reload,scalar_dynamic_offset,"
            "vector_dynamic_offsets,dst_reduce,transpose",
        ]

    bu.get_walrus_args = patched
    bu._agnn_dge_patch = True


def _split_multi_waits(nc):
    import bass_rust
    import concourse.mybir as mybir

    cnt = 0
    for func in nc.m.functions:
        for block in func.blocks:
            out = []
            for inst in block.instructions:
                si = inst.sync_info
                cap = 2 if isinstance(inst, mybir.InstEventSemaphore) else 1
                if (si is not None and si.on_wait and len(si.on_wait) > cap
                        and inst.engine is not None):
                    waits = list(si.on_wait)
                    for w in waits[cap:]:
                        cnt += 1
                        nop = mybir.InstNoOp(
                            name=f"wsplit{cnt}", engine=inst.engine, ins=[], outs=[]
                        )
                        nop.sync_info = bass_rust.SyncInfo(on_wait=[w], on_update=[])
                        try:
                            nc.register_instruction(nop, overwrite=True)
                        except Exception:
                            pass
                        out.append(nop)
                    inst.sync_info = bass_rust.SyncInfo(
                        on_wait=waits[:cap], on_update=list(si.on_update or [])
                    )
                out.append(inst)
            try:
                block.instructions = out
            except Exception:
                block.instructions.clear()
                block.instructions.extend(out)
    return cnt


def _build_nc(cfg):
    _patch_walrus_args()
    import concourse.bacc as bacc
    import concourse.mybir as mybir
    import concourse.tile as tile
    from concourse.masks import make_identity

    K = cfg["K"]; TOTC = cfg["TOTC"]
    groups = cfg["groups"]; batches = cfg["batches"]

    f32 = mybir.dt.float32
    bf16 = mybir.dt.bfloat16

    nc = bacc.Bacc("TRN2", num_devices=CORES, dynamic_dma_scratch_size=65536)

    xT_t = nc.declare_dram_parameter("xT", [IN, NPP], bf16, isOutput=False)
    idx_t = nc.declare_dram_parameter("idx", [P, TOTC * 8], mybir.dt.int16, isOutput=False)
    sidx_t = nc.declare_dram_parameter("sidx", [NCH, P, NPP // 16], mybir.dt.int16, isOutput=False)
    W1_t = nc.declare_dram_parameter("W1", [IN, HID], bf16, isOutput=False)
    b1_t = nc.declare_dram_parameter("b1", [HID, 1], f32, isOutput=False)
    W2_t = nc.declare_dram_parameter("W2", [HID, HID], bf16, isOutput=False)
    b2_t = nc.declare_dram_parameter("b2", [HID, 1], f32, isOutput=False)
    beta_t = nc.declare_dram_parameter("beta128", [P, 1], f32, isOutput=False)
    agg_t = nc.declare_dram_parameter("agg", [NCH, P, NB, HID], bf16, isOutput=True)
    s_t = nc.declare_dram_parameter("s", [NCH, P, NB], f32, isOutput=True)
    sagg_t = nc.declare_dram_parameter("selfagg", [P, NB, HID], f32, isOutput=True)

    with tile.TileContext(nc) as tc:
        agin, free_agin = tc.tile([P, NB, D2], bf16, space="DRAM", name="agin")
        import contextlib
        _tabctx = contextlib.ExitStack()
        _tabpool = _tabctx.enter_context(
            tc.tile_pool(name="tabpool", space="DRAM", bufs=1))
        table = _tabpool.tile([CORES * NPP, D2], bf16, name="table",
                              addr_space="Shared")

        def free_table():
            _tabctx.close()

        with tc.tile_pool(name="consts", bufs=1) as cpool:
            W1sb = cpool.tile([IN, HID], bf16)
            W2sb = cpool.tile([HID, HID], bf16)
            b1sb = cpool.tile([HID, 1], f32)
            b2sb = cpool.tile([HID, 1], f32)
            betasb = cpool.tile([P, 1], f32)
            sbeta = cpool.tile([P, 1], f32)
            ident = cpool.tile([P, P], f32)
            identb = cpool.tile([P, P], bf16)
            hTD = cpool.tile([P, NB, HID], bf16)
            zrow = cpool.tile([1, 1, D2], bf16)
            hnD = [cpool.tile([P, NB, HID], bf16, name=f"hnD{c}") for c in range(NCH)]

            nc.sync.dma_start(out=W1sb[:], in_=W1_t[:])
            nc.sync.dma_start(out=W2sb[:], in_=W2_t[:])
            nc.sync.dma_start(out=b1sb[:], in_=b1_t[:])
            nc.sync.dma_start(out=b2sb[:], in_=b2_t[:])
            nc.sync.dma_start(out=betasb[:], in_=beta_t[:])
            make_identity(nc, ident[:])
            nc.vector.tensor_copy(identb[:], ident[:])
            nc.scalar.activation(sbeta[:], betasb[:], mybir.ActivationFunctionType.Exp)
            nc.gpsimd.memset(zrow[:], 0)
            stile = [cpool.tile([P, NB], f32, name=f"stile{c}") for c in range(NCH)]

            # ---------------- phase 1: MLP + packed local table ------------
            with (
                tc.tile_pool(name="mlp", bufs=3) as mpool,
                tc.tile_pool(name="mlpp", bufs=2, space="PSUM") as mpp,
            ):
                for it in range(NIT):
                    nbk = min(4, NB - 4 * it)
                    cols = nbk * P
                    s0 = it * 4 * P
                    xt = mpool.tile([IN, 512], bf16, tag="xt")
                    nc.sync.dma_start(out=xt[:, 0:cols], in_=xT_t[:, s0:s0 + cols])
                    ps1 = mpp.tile([HID, 512], f32, tag="ps1")
                    nc.tensor.matmul(ps1[:, 0:cols], lhsT=W1sb[:], rhs=xt[:, 0:cols],
                                     start=True, stop=True)
                    r1 = mpool.tile([HID, 512], bf16, tag="r1")
                    nc.scalar.activation(r1[:, 0:cols], ps1[:, 0:cols],
                                         mybir.ActivationFunctionType.Relu, bias=b1sb[:])
                    ps2 = mpp.tile([HID, 512], f32, tag="ps2")
                    nc.tensor.matmul(ps2[:, 0:cols], lhsT=W2sb[:], rhs=r1[:, 0:cols],
                                     start=True, stop=True)
                    hb = mpool.tile([HID, 512], bf16, tag="hb")
                    nc.scalar.activation(hb[:, 0:cols], ps2[:, 0:cols],
                                         mybir.ActivationFunctionType.Identity, bias=b2sb[:])
                    pst = mpp.tile([P, 4, HID], bf16, tag="pst")
                    for j in range(nbk):
                        nc.tensor.transpose(pst[:, j, :], hb[:, j * P:(j + 1) * P],
                                            identb[0:HID, 0:HID])
                    hts = hTD[:, 4 * it:4 * it + nbk, :]
                    nc.scalar.activation(hts, pst[:, 0:nbk, :],
                                         mybir.ActivationFunctionType.Copy)
                    sq = mpool.tile([P, 4, HID], bf16, tag="sq")
                    nc.vector.tensor_tensor(out=sq[:, 0:nbk, :], in0=hts, in1=hts,
                                            op=mybir.AluOpType.mult)
                    ss = mpool.tile([P, 4], f32, tag="ss")
                    nc.vector.tensor_reduce(out=ss[:, 0:nbk], in_=sq[:, 0:nbk, :],
                                            axis=mybir.AxisListType.X,
                                            op=mybir.AluOpType.add)
                    nrm = mpool.tile([P, 4], f32, tag="nrm")
                    nc.scalar.activation(nrm[:, 0:nbk], ss[:, 0:nbk],
                                         mybir.ActivationFunctionType.Sqrt)
                    nc.vector.tensor_scalar_max(out=nrm[:, 0:nbk], in0=nrm[:, 0:nbk],
                                                scalar1=1e-12)
                    rn = mpool.tile([P, 4], f32, tag="rn")
                    nc.vector.reciprocal(rn[:, 0:nbk], nrm[:, 0:nbk])
                    tabt = mpool.tile([P, 4, D2], bf16, tag="tabt")
                    nc.vector.tensor_tensor(
                        out=tabt[:, 0:nbk, 0:HID], in0=hts,
                        in1=rn[:, 0:nbk].unsqueeze(2).to_broadcast([P, nbk, HID]),
                        op=mybir.AluOpType.mult)
                    nc.vector.tensor_copy(tabt[:, 0:nbk, HID:D2], hts)
                    nc.sync.dma_start(out=agin[:, 4 * it:4 * it + nbk, :],
                                      in_=tabt[:, 0:nbk, :])
            # zero row for gather pads — AFTER the MLP stores (overwrites the
            # fake node at (p=127, b=97) = flat row 12543)
            nc.sync.dma_start(out=agin[P - 1:P, NB - 1:NB, :], in_=zrow[:])

            # self-loop partial (base order): exp(beta) * h
            with tc.tile_pool(name="selfp", bufs=1) as spool_:
                sagg = spool_.tile([P, NB, HID], f32)
                nc.vector.tensor_scalar_mul(out=sagg[:], in0=hTD[:], scalar1=sbeta[:])
                nc.sync.dma_start(out=sagg_t[:], in_=sagg[:])

            # ---------------- phase 2: AllGather the table -----------------
            nc.gpsimd.collective_compute(
                "AllGather",
                mybir.AluOpType.bypass,
                replica_groups=[list(range(CORES))],
                ins=[agin[:].flatten_outer_dims().opt()],
                outs=[table[:].opt()],
            )

            # ---------------- phase 2.5: per-chunk dst tables --------------
            with tc.tile_pool(name="hnprep", bufs=1) as hpool:
                aginflat = agin[:].flatten_outer_dims()
                for c in range(NCH):
                    sxi = hpool.tile([P, NPP // 16], mybir.dt.int16, tag="sxi")
                    nc.sync.dma_start(out=sxi[:], in_=sidx_t[c])
                    scr = hpool.tile([P, NB, D2], bf16, tag="scr")
                    nc.gpsimd.dma_gather(scr[:], aginflat, sxi[:], NPP, NPP, D2,
                                         single_packet=False)
                    nc.vector.tensor_copy(hnD[c][:], scr[:, :, 0:HID])

            # ---------------- phase 3: batched per-group AGNN partials -----
            # Stage-staggered: s1(dots+exp) for group g, s2(weights+PE) for
            # g-1, s3(psum copy+store) for g-2.
            with (
                tc.tile_pool(name="blk", bufs=5) as bpool,
                tc.tile_pool(name="blks", bufs=2) as spool,
                tc.tile_pool(name="sacc", bufs=2) as apool,
                tc.tile_pool(name="blkp", bufs=3, space="PSUM") as bpp,
            ):
                ngr = [len(groups[c]) for c in range(NCH)]
                done = [0] * NCH

                def s2(e):
                    c = e["c"]; nbk = e["nbk"]; Kg = e["Kg"]; nk = nbk * Kg
                    nc.vector.tensor_reduce(
                        out=stile[c][:, e["b0"]:e["b0"] + nbk], in_=e["ex2"][:],
                        axis=mybir.AxisListType.XY, op=mybir.AluOpType.add)
                    wts = spool.tile([P, nbk, Kg, HID], bf16, tag="wts")
                    nc.vector.tensor_tensor(
                        out=wts[:].rearrange("p n k (a b) -> p (n k) a b", b=2),
                        in0=e["slh"].rearrange("p c (a b) -> p c a b", b=2),
                        in1=e["ex2"][:].rearrange("p n k e -> p (n k) e")
                            .unsqueeze(2).to_broadcast([P, nk, 32, 2]),
                        op=mybir.AluOpType.mult)
                    pgrp = bpp.tile([P, MAXG, HID], f32, tag="pgrp")
                    for k in range(Kg):
                        nc.tensor.matmul(pgrp[:, 0:nbk, :], lhsT=identb[:],
                                         rhs=wts[:, :, k, :],
                                         start=(k == 0), stop=(k == Kg - 1))
                    e["pgrp"] = pgrp

                def s3(e):
                    c = e["c"]; nbk = e["nbk"]
                    aggsb = apool.tile([P, MAXG, HID], bf16, tag="aggsb")
                    nc.scalar.activation(aggsb[:, 0:nbk, :], e["pgrp"][:, 0:nbk, :],
                                         mybir.ActivationFunctionType.Copy)
                    nc.sync.dma_start(out=agg_t[c, :, e["b0"]:e["b0"] + nbk, :],
                                      in_=aggsb[:, 0:nbk, :])
                    done[c] += 1
                    if done[c] == ngr[c]:
                        nc.sync.dma_start(out=s_t[c], in_=stile[c][:])

                prev1 = None
                prev2 = None
                with nc.allow_low_precision(reason="bf16 edge logits"):
                    for c in range(NCH):
                        for (col0, ncols, g0, ng) in batches[c]:
                            wc0, wcn = col0 * 8, ncols * 8
                            idxsb = bpool.tile([P, BATCH_COLS * 8],
                                               mybir.dt.int16, tag="idx")
                            nc.sync.dma_start(out=idxsb[:, 0:wcn],
                                              in_=idx_t[:, wc0:wc0 + wcn])
                            slots = bpool.tile([P, BATCH_COLS, D2], bf16,
                                               tag="slots")
                            nc.gpsimd.dma_gather(
                                slots[:, 0:ncols, :],
                                table[c * CHROWS:(c + 1) * CHROWS, :],
                                idxsb[:, 0:wcn],
                                ncols * P, ncols * P, D2, single_packet=False)
                            for gi in range(g0, g0 + ng):
                                (b0, nbk, Kg, gcol0) = groups[c][gi]
                                cb = gcol0 - col0
                                nk = nbk * Kg
                                slh = slots[:, cb:cb + nk, HID:D2]
                                sl4 = slots[:, cb:cb + nk, 0:HID].rearrange(
                                    "p (n k) e -> p n k e", k=Kg)
                                tmp = spool.tile([P, nbk, Kg, HID], bf16,
                                                 tag="tmp")
                                nc.vector.tensor_tensor(
                                    out=tmp[:], in0=sl4,
                                    in1=hnD[c][:, b0:b0 + nbk, :].unsqueeze(2)
                                        .to_broadcast([P, nbk, Kg, HID]),
                                    op=mybir.AluOpType.mult)
                                delta = spool.tile([P, nbk, Kg], bf16,
                                                   tag="delta")
                                nc.vector.tensor_reduce(
                                    out=delta[:], in_=tmp[:],
                                    axis=mybir.AxisListType.X,
                                    op=mybir.AluOpType.add)
                                ex2 = spool.tile([P, nbk, Kg, 2], bf16,
                                                 tag="ex2")
                                nc.scalar.activation(
                                    ex2[:],
                                    delta[:].unsqueeze(3)
                                        .to_broadcast([P, nbk, Kg, 2]),
                                    mybir.ActivationFunctionType.Exp,
                                    scale=betasb[:])
                                e = dict(c=c, b0=b0, nbk=nbk, Kg=Kg,
                                         ex2=ex2, slh=slh)
                                if prev1 is not None:
                                    s2(prev1)
                                if prev2 is not None:
                                    s3(prev2)
                                prev2, prev1 = prev1, e
                    if prev1 is not None:
                        s2(prev1)
                    if prev2 is not None:
                        s3(prev2)
                    if prev1 is not None:
                        s3(prev1)

        free_agin()
        free_table()

    nc.compile()
    _split_multi_waits(nc)
    return nc


# ------------------------------------------------------------------- driver
_CACHE = {}


def _get_nc(cfg_key, cfg):
    if cfg_key not in _CACHE:
        _CACHE[cfg_key] = _build_nc(cfg)
    return _CACHE[cfg_key]


def _make_in_maps(inputs, prep):
    x = np.asarray(inputs["x"], dtype=np.float32)
    W1 = np.asarray(inputs["W1"], dtype=np.float32)
    b1 = np.asarray(inputs["b1"], dtype=np.float32).reshape(-1, 1)
    W2 = np.asarray(inputs["W2"], dtype=np.float32)
    b2 = np.asarray(inputs["b2"], dtype=np.float32).reshape(-1, 1)
    beta = np.asarray(inputs["beta"], dtype=np.float32)
    beta128 = np.repeat(beta.reshape(1, 1), P, axis=0).astype(np.float32)
    import ml_dtypes
    bf = ml_dtypes.bfloat16

    in_maps = []
    for p in range(CORES):
        xp = x[p * NP:(p + 1) * NP]
        xpad = np.zeros((NPP, IN), np.float32)
        xpad[:NP] = xp
        in_maps.append({
            "xT": np.ascontiguousarray(xpad.T).astype(bf),
            "idx": prep["idx_all"][p],
            "sidx": prep["sidx_all"][p],
            "W1": W1.astype(bf), "b1": b1,
            "W2": W2.astype(bf), "b2": b2,
            "beta128": beta128,
        })
    return in_maps


def _postprocess(results, prep, inputs):
    Wc = np.asarray(inputs["Wc"], dtype=np.float64)
    bc = np.asarray(inputs["bc"], dtype=np.float64)
    beta = float(np.asarray(inputs["beta"]).reshape(-1)[0])
    expb = math.exp(beta)
    K = prep["Keff"]
    y = np.zeros((N, Wc.shape[1]), np.float32)
    n = np.arange(NP)
    bb, dd = n // P, n % P
    for p in range(CORES):
        res = results[p]
        agg = res["agg"].astype(np.float64)       # [4, 128, 98, 64]
        sdev = res["s"].astype(np.float64)        # [4, 128, 98]
        sagg = res["selfagg"].astype(np.float64)  # [128, 98, 64]
        acc = sagg[dd, bb, :].copy()              # self term, base order
        stot = np.full(NP, expb)
        cntp = prep["cnt"][p * NP:(p + 1) * NP]
        for c in range(NCH):
            pos = prep["pos_pc"][p, c]
            b_, d_ = pos // P, pos % P
            acc += agg[c, d_, b_, :]
            stot += sdev[c, d_, b_] / 2.0 - (K[c, b_] - cntp[:, c])
        y[p * NP:(p + 1) * NP] = ((acc / stot[:, None]) @ Wc + bc).astype(np.float32)
    return y


def get_nc_for_test(inputs):
    prep = _preprocess(np.asarray(inputs["edge_index"]))
    cfg = dict(K=prep["K"], TOTC=prep["TOTC"], groups=prep["groups"], batches=prep["batches"])
    cfg_key = ("v6", tuple(int(k) for k in prep["Keff"].ravel()))
    return _get_nc(cfg_key, cfg)


def kernel(**inputs):
    from concourse.bass_utils import run_bass_kernel_spmd

    prep = _preprocess(np.asarray(inputs["edge_index"]))
    cfg = dict(K=prep["K"], TOTC=prep["TOTC"], groups=prep["groups"], batches=prep["batches"])
    cfg_key = ("v6", tuple(int(k) for k in prep["Keff"].ravel()))
    nc = _get_nc(cfg_key, cfg)
    in_maps = _make_in_maps(inputs, prep)
    res = run_bass_kernel_spmd(nc, in_maps, core_ids=list(range(CORES)))
    return _postprocess(res.results, prep, inputs)


if __name__ == "__main__":
    pass



# revision 7
# speedup vs baseline: 1.1046x; 1.0237x over previous
"""AGNN (AMNet) message-passing kernel for 8 TRN2 NeuronCores.

Design (vs the v1 baseline):
  - Nodes partitioned contiguously across 8 cores (no global sort).
  - Phase 1: input MLP in bf16, 4 blocks (512 nodes) per iteration; packed
    [hn|h] bf16 rows stored to a local HBM table `agin` in partition-major
    layout (node n -> flat row (n%128)*98 + n//128).
  - Phase 2: ONE AllGather replicates the table (addr_space Shared).
  - Per-chunk independent dst orderings: for each of the 4 table chunks
    (int16 gather reach), each core re-sorts its OWN dsts by that chunk's
    in-degree, making the per-(block,chunk) rectangle schedule tight
    (~1.1x padding vs 2.07x for a common ordering).
  - Device outputs per-chunk partial aggregates (fp32) and exp-sums; the
    softmax combine across chunks + self-loop term + classifier run on the
    HOST (they commute with the chunk decomposition).
  - No tile_critical / manual semaphores: tile auto-tracks dma_gather.

kernel() accepts FULL inputs and returns the FULL [N, 2] float32 output.
"""

import math
import os
import sys

sys.path.insert(0, "/opt/trn_rl_repo")

import numpy as np

CORES = 8
P = 128
NCH = 4
N = 100000
NP = N // CORES            # 12500
NB = math.ceil(NP / P)     # 98
NPP = NB * P               # 12544
CHROWS = 2 * NPP           # 25088
ZROW = NPP - 1             # 12543  == (12543%128)*98 + 12543//128
SBB = 8                    # blocks per super-block (one gather each)
GB = SBB                   # psum->sbuf copy group
IN, HID = 128, 64
D2 = 2 * HID
NIT = NB // 4 + (1 if NB % 4 else 0)  # 25 MLP iterations (4 blocks each)

AGG_DVE = bool(os.environ.get("AGG_DVE"))  # A/B: aggregation on DVE vs PE
MAXG = 8                       # max blocks per compute group (PSUM)
MAXGC = 48                     # max columns per compute group (SBUF)
BATCH_COLS = 48                # gather batch size (columns)
CCOL = 330.0                   # DP cost per padded column (ns-ish)
CGRP = 1500.0                  # DP cost per extra group


def _dp_groups(Kc):
    """Consecutive groups (<=MAXG blocks, <=MAXGC cols) minimizing
    sum(len*Kmax*CCOL + CGRP)."""
    nb = len(Kc)
    best = [float("inf")] * (nb + 1)
    prev = [0] * (nb + 1)
    best[0] = 0.0
    for i in range(1, nb + 1):
        kmax = 0
        for L in range(1, MAXG + 1):
            j = i - L
            if j < 0:
                break
            kmax = max(kmax, Kc[j])
            if L * kmax > MAXGC:
                break
            cst = best[j] + L * kmax * CCOL + CGRP
            if cst < best[i]:
                best[i] = cst
                prev[i] = j
    out = []
    i = nb
    while i > 0:
        j = prev[i]
        out.append((j, i - j, int(max(Kc[j:i]))))
        i = j
    return out[::-1]


def _flatrow(n):
    """Local table row of local node id n (partition-major layout)."""
    return (n % P) * NB + n // P


def _wrap_idx(flat):
    n = flat.shape[0]
    w = flat.reshape(n // 16, 16).T
    return np.tile(w, (8, 1)).astype(np.int16)


# ----------------------------------------------------------------- host prep
def _preprocess(edge_index):
    src = np.asarray(edge_index[0], dtype=np.int64)
    dst = np.asarray(edge_index[1], dtype=np.int64)
    chunk_of_src = src // (2 * NP)

    # CSR sorted by (dst, chunk)
    order = np.lexsort((src, chunk_of_src, dst))
    src_s = src[order]
    ch_s = chunk_of_src[order]
    # per (dst, chunk) counts
    cnt = np.zeros((N, NCH), np.int64)
    np.add.at(cnt, (dst, chunk_of_src), 1)
    cnt_cum = np.concatenate(
        [np.zeros((N, 1), np.int64), np.cumsum(cnt, axis=1)], axis=1
    )  # [N, 5]
    deg = cnt.sum(axis=1)
    row_start = np.zeros(N + 1, np.int64)
    np.cumsum(deg, out=row_start[1:])

    # local table row of each src (within its chunk)
    q = src_s // NP
    nloc = src_s % NP
    locrow = (q % 2) * NPP + (nloc % P) * NB + nloc // P  # within-chunk row

    # per-(core, chunk) orderings + per-block maxima
    pos_pc = np.zeros((CORES, NCH, NP), np.int64)   # node -> position
    Kpc = np.zeros((CORES, NCH, NB), np.int64)
    for p in range(CORES):
        lo = p * NP
        cl = cnt[lo:lo + NP]                         # [NP, NCH]
        for c in range(NCH):
            o = np.argsort(-cl[:, c], kind="stable")
            pos = np.empty(NP, np.int64)
            pos[o] = np.arange(NP)
            pos_pc[p, c] = pos
            srt = cl[o, c]
            padded = np.zeros(NPP, np.int64)
            padded[:NP] = srt
            Kpc[p, c] = padded.reshape(NB, P).max(axis=1)
    K = Kpc.max(axis=0)                              # [NCH, NB]
    K = np.maximum(K, 1)

    # flexible uniform-K groups per chunk + gather batches of whole groups
    colstart = np.zeros((NCH, NB), np.int64)
    Keff = np.zeros((NCH, NB), np.int64)
    groups = []   # [NCH] list of (b0, nbk, Kg, col0)  (col0 global)
    batches = []  # [NCH] list of (col0, ncols, g0, ng)
    coff = 0
    for c in range(NCH):
        gs = _dp_groups(list(K[c]))
        glist = []
        bl = []
        bc0, bg0, bnc = coff, 0, 0
        for gi, (b0, nbk, Kg) in enumerate(gs):
            w = nbk * Kg
            if bnc + w > BATCH_COLS and bnc > 0:
                bl.append((bc0, bnc, bg0, gi - bg0))
                bc0, bg0, bnc = coff, gi, 0
            glist.append((b0, nbk, Kg, coff))
            Keff[c, b0:b0 + nbk] = Kg
            for j in range(nbk):
                colstart[c, b0 + j] = coff + j * Kg
            coff += w
            bnc += w
        bl.append((bc0, bnc, bg0, len(gs) - bg0))
        groups.append(glist)
        batches.append(bl)
    TOTC = coff

    # index streams
    idx_all = np.zeros((CORES, P, TOTC * 8), np.int16)
    sidx_all = np.zeros((CORES, NCH, P, NPP // 16), np.int16)
    for p in range(CORES):
        lo = p * NP
        e0, e1 = row_start[lo], row_start[lo + NP]
        d_e = dst[order[e0:e1]] - lo
        c_e = ch_s[e0:e1]
        r_e = locrow[e0:e1]
        # k-rank of each edge within its (dst, chunk) segment
        seg0 = row_start[d_e + lo] - e0 + cnt_cum[d_e + lo, c_e]
        k_e = np.arange(e1 - e0) - seg0
        A = np.full((TOTC, P), ZROW, np.int16)
        pos_e = pos_pc[p, c_e, d_e]
        col_e = colstart[c_e, pos_e // P] + k_e
        A[col_e, pos_e % P] = r_e.astype(np.int16)
        idx_all[p] = _wrap_idx(A.ravel())
        for c in range(NCH):
            o = np.argsort(pos_pc[p, c], kind="stable")  # position -> node
            rows = np.full(NPP, ZROW, np.int64)
            rows[:NP] = _flatrow(o)
            sidx_all[p, c] = _wrap_idx(rows)

    return dict(K=K, Keff=Keff, TOTC=TOTC, groups=groups, batches=batches,
                colstart=colstart,
                pos_pc=pos_pc, cnt=cnt, idx_all=idx_all, sidx_all=sidx_all)


# ------------------------------------------------------------------ builder
def _patch_walrus_args():
    import concourse.bass_utils as bu
    if getattr(bu, "_agnn_dge_patch", False):
        return
    orig = bu.get_walrus_args

    def patched(*a, **k):
        return list(orig(*a, **k)) + [
            "--dge-levels=io,spill_reload,scalar_dynamic_offset,"
            "vector_dynamic_offsets,dst_reduce,transpose",
        ]

    bu.get_walrus_args = patched
    bu._agnn_dge_patch = True


def _split_multi_waits(nc):
    import bass_rust
    import concourse.mybir as mybir

    cnt = 0
    for func in nc.m.functions:
        for block in func.blocks:
            out = []
            for inst in block.instructions:
                si = inst.sync_info
                cap = 2 if isinstance(inst, mybir.InstEventSemaphore) else 1
                if (si is not None and si.on_wait and len(si.on_wait) > cap
                        and inst.engine is not None):
                    waits = list(si.on_wait)
                    for w in waits[cap:]:
                        cnt += 1
                        nop = mybir.InstNoOp(
                            name=f"wsplit{cnt}", engine=inst.engine, ins=[], outs=[]
                        )
                        nop.sync_info = bass_rust.SyncInfo(on_wait=[w], on_update=[])
                        try:
                            nc.register_instruction(nop, overwrite=True)
                        except Exception:
                            pass
                        out.append(nop)
                    inst.sync_info = bass_rust.SyncInfo(
                        on_wait=waits[:cap], on_update=list(si.on_update or [])
                    )
                out.append(inst)
            try:
                block.instructions = out
            except Exception:
                block.instructions.clear()
                block.instructions.extend(out)
    return cnt


def _build_nc(cfg):
    _patch_walrus_args()
    import concourse.bacc as bacc
    import concourse.mybir as mybir
    import concourse.tile as tile
    from concourse.masks import make_identity

    K = cfg["K"]; TOTC = cfg["TOTC"]
    groups = cfg["groups"]; batches = cfg["batches"]

    f32 = mybir.dt.float32
    bf16 = mybir.dt.bfloat16

    nc = bacc.Bacc("TRN2", num_devices=CORES, dynamic_dma_scratch_size=65536)

    xT_t = nc.declare_dram_parameter("xT", [IN, NPP], bf16, isOutput=False)
    idx_t = nc.declare_dram_parameter("idx", [P, TOTC * 8], mybir.dt.int16, isOutput=False)
    sidx_t = nc.declare_dram_parameter("sidx", [NCH, P, NPP // 16], mybir.dt.int16, isOutput=False)
    W1_t = nc.declare_dram_parameter("W1", [IN, HID], bf16, isOutput=False)
    b1_t = nc.declare_dram_parameter("b1", [HID, 1], f32, isOutput=False)
    W2_t = nc.declare_dram_parameter("W2", [HID, HID], bf16, isOutput=False)
    b2_t = nc.declare_dram_parameter("b2", [HID, 1], f32, isOutput=False)
    beta_t = nc.declare_dram_parameter("beta128", [P, 1], f32, isOutput=False)
    agg_t = nc.declare_dram_parameter("agg", [NCH, P, NB, HID], bf16, isOutput=True)
    s_t = nc.declare_dram_parameter("s", [NCH, P, NB], f32, isOutput=True)
    sagg_t = nc.declare_dram_parameter("selfagg", [P, NB, HID], f32, isOutput=True)

    with tile.TileContext(nc) as tc:
        agin, free_agin = tc.tile([P, NB, D2], bf16, space="DRAM", name="agin")
        import contextlib
        _tabctx = contextlib.ExitStack()
        _tabpool = _tabctx.enter_context(
            tc.tile_pool(name="tabpool", space="DRAM", bufs=1))
        table = _tabpool.tile([CORES * NPP, D2], bf16, name="table",
                              addr_space="Shared")

        def free_table():
            _tabctx.close()

        with tc.tile_pool(name="consts", bufs=1) as cpool:
            W1sb = cpool.tile([IN, HID], bf16)
            W2sb = cpool.tile([HID, HID], bf16)
            b1sb = cpool.tile([HID, 1], f32)
            b2sb = cpool.tile([HID, 1], f32)
            betasb = cpool.tile([P, 1], f32)
            sbeta = cpool.tile([P, 1], f32)
            ident = cpool.tile([P, P], f32)
            identb = cpool.tile([P, P], bf16)
            hTD = cpool.tile([P, NB, HID], bf16)
            zrow = cpool.tile([1, 1, D2], bf16)
            hnD = [cpool.tile([P, NB, HID], bf16, name=f"hnD{c}") for c in range(NCH)]

            nc.sync.dma_start(out=W1sb[:], in_=W1_t[:])
            nc.sync.dma_start(out=W2sb[:], in_=W2_t[:])
            nc.sync.dma_start(out=b1sb[:], in_=b1_t[:])
            nc.sync.dma_start(out=b2sb[:], in_=b2_t[:])
            nc.sync.dma_start(out=betasb[:], in_=beta_t[:])
            make_identity(nc, ident[:])
            nc.vector.tensor_copy(identb[:], ident[:])
            nc.scalar.activation(sbeta[:], betasb[:], mybir.ActivationFunctionType.Exp)
            nc.gpsimd.memset(zrow[:], 0)
            stile = [cpool.tile([P, NB], f32, name=f"stile{c}") for c in range(NCH)]

            # ---------------- phase 1: MLP + packed local table ------------
            with (
                tc.tile_pool(name="mlp", bufs=3) as mpool,
                tc.tile_pool(name="mlpp", bufs=2, space="PSUM") as mpp,
            ):
                for it in range(NIT):
                    nbk = min(4, NB - 4 * it)
                    cols = nbk * P
                    s0 = it * 4 * P
                    xt = mpool.tile([IN, 512], bf16, tag="xt")
                    nc.sync.dma_start(out=xt[:, 0:cols], in_=xT_t[:, s0:s0 + cols])
                    ps1 = mpp.tile([HID, 512], f32, tag="ps1")
                    nc.tensor.matmul(ps1[:, 0:cols], lhsT=W1sb[:], rhs=xt[:, 0:cols],
                                     start=True, stop=True)
                    r1 = mpool.tile([HID, 512], bf16, tag="r1")
                    nc.scalar.activation(r1[:, 0:cols], ps1[:, 0:cols],
                                         mybir.ActivationFunctionType.Relu, bias=b1sb[:])
                    ps2 = mpp.tile([HID, 512], f32, tag="ps2")
                    nc.tensor.matmul(ps2[:, 0:cols], lhsT=W2sb[:], rhs=r1[:, 0:cols],
                                     start=True, stop=True)
                    hb = mpool.tile([HID, 512], bf16, tag="hb")
                    nc.scalar.activation(hb[:, 0:cols], ps2[:, 0:cols],
                                         mybir.ActivationFunctionType.Identity, bias=b2sb[:])
                    pst = mpp.tile([P, 4, HID], bf16, tag="pst")
                    for j in range(nbk):
                        nc.tensor.transpose(pst[:, j, :], hb[:, j * P:(j + 1) * P],
                                            identb[0:HID, 0:HID])
                    hts = hTD[:, 4 * it:4 * it + nbk, :]
                    nc.scalar.activation(hts, pst[:, 0:nbk, :],
                                         mybir.ActivationFunctionType.Copy)
                    sq = mpool.tile([P, 4, HID], bf16, tag="sq")
                    nc.vector.tensor_tensor(out=sq[:, 0:nbk, :], in0=hts, in1=hts,
                                            op=mybir.AluOpType.mult)
                    ss = mpool.tile([P, 4], f32, tag="ss")
                    nc.vector.tensor_reduce(out=ss[:, 0:nbk], in_=sq[:, 0:nbk, :],
                                            axis=mybir.AxisListType.X,
                                            op=mybir.AluOpType.add)
                    nrm = mpool.tile([P, 4], f32, tag="nrm")
                    nc.scalar.activation(nrm[:, 0:nbk], ss[:, 0:nbk],
                                         mybir.ActivationFunctionType.Sqrt)
                    nc.vector.tensor_scalar_max(out=nrm[:, 0:nbk], in0=nrm[:, 0:nbk],
                                                scalar1=1e-12)
                    rn = mpool.tile([P, 4], f32, tag="rn")
                    nc.vector.reciprocal(rn[:, 0:nbk], nrm[:, 0:nbk])
                    tabt = mpool.tile([P, 4, D2], bf16, tag="tabt")
                    nc.vector.tensor_tensor(
                        out=tabt[:, 0:nbk, 0:HID], in0=hts,
                        in1=rn[:, 0:nbk].unsqueeze(2).to_broadcast([P, nbk, HID]),
                        op=mybir.AluOpType.mult)
                    nc.vector.tensor_copy(tabt[:, 0:nbk, HID:D2], hts)
                    nc.sync.dma_start(out=agin[:, 4 * it:4 * it + nbk, :],
                                      in_=tabt[:, 0:nbk, :])
            # zero row for gather pads — AFTER the MLP stores (overwrites the
            # fake node at (p=127, b=97) = flat row 12543)
            nc.sync.dma_start(out=agin[P - 1:P, NB - 1:NB, :], in_=zrow[:])

            # self-loop partial (base order): exp(beta) * h
            with tc.tile_pool(name="selfp", bufs=1) as spool_:
                sagg = spool_.tile([P, NB, HID], f32)
                nc.vector.tensor_scalar_mul(out=sagg[:], in0=hTD[:], scalar1=sbeta[:])
                nc.sync.dma_start(out=sagg_t[:], in_=sagg[:])

            # ---------------- phase 2: AllGather the table -----------------
            nc.gpsimd.collective_compute(
                "AllGather",
                mybir.AluOpType.bypass,
                replica_groups=[list(range(CORES))],
                ins=[agin[:].flatten_outer_dims().opt()],
                outs=[table[:].opt()],
            )

            # ---------------- phase 2.5: per-chunk dst tables --------------
            with tc.tile_pool(name="hnprep", bufs=1) as hpool:
                aginflat = agin[:].flatten_outer_dims()
                for c in range(NCH):
                    sxi = hpool.tile([P, NPP // 16], mybir.dt.int16, tag="sxi")
                    nc.sync.dma_start(out=sxi[:], in_=sidx_t[c])
                    scr = hpool.tile([P, NB, D2], bf16, tag="scr")
                    nc.gpsimd.dma_gather(scr[:], aginflat, sxi[:], NPP, NPP, D2,
                                         single_packet=False)
                    nc.vector.tensor_copy(hnD[c][:], scr[:, :, 0:HID])

            # ---------------- phase 3: batched per-group AGNN partials -----
            # Stage-staggered: s1(dots+exp) for group g, s2(weights+PE) for
            # g-1, s3(psum copy+store) for g-2.
            with (
                tc.tile_pool(name="blk", bufs=5) as bpool,
                tc.tile_pool(name="blks", bufs=2) as spool,
                tc.tile_pool(name="sacc", bufs=7) as apool,
                tc.tile_pool(name="blkp", bufs=8, space="PSUM") as bpp,
            ):
                ngr = [len(groups[c]) for c in range(NCH)]
                done = [0] * NCH

                def s2(e):
                    c = e["c"]; nbk = e["nbk"]; Kg = e["Kg"]; nk = nbk * Kg
                    nc.vector.tensor_reduce(
                        out=stile[c][:, e["b0"]:e["b0"] + nbk], in_=e["ex2"][:],
                        axis=mybir.AxisListType.XY, op=mybir.AluOpType.add)
                    wts = spool.tile([P, nbk, Kg, HID], bf16, tag="wts")
                    nc.vector.tensor_tensor(
                        out=wts[:].rearrange("p n k (a b) -> p (n k) a b", b=2),
                        in0=e["slh"].rearrange("p c (a b) -> p c a b", b=2),
                        in1=e["ex2"][:].rearrange("p n k e -> p (n k) e")
                            .unsqueeze(2).to_broadcast([P, nk, 32, 2]),
                        op=mybir.AluOpType.mult)
                    pgrp = bpp.tile([P, MAXG, HID], f32, tag="pgrp")
                    for k in range(Kg):
                        nc.tensor.matmul(pgrp[:, 0:nbk, :], lhsT=identb[:],
                                         rhs=wts[:, :, k, :],
                                         start=(k == 0), stop=(k == Kg - 1))
                    e["pgrp"] = pgrp

                def s3(e):
                    c = e["c"]; nbk = e["nbk"]
                    aggsb = apool.tile([P, MAXG, HID], bf16, tag="aggsb")
                    nc.scalar.activation(aggsb[:, 0:nbk, :], e["pgrp"][:, 0:nbk, :],
                                         mybir.ActivationFunctionType.Copy)
                    nc.sync.dma_start(out=agg_t[c, :, e["b0"]:e["b0"] + nbk, :],
                                      in_=aggsb[:, 0:nbk, :])
                    done[c] += 1
                    if done[c] == ngr[c]:
                        nc.sync.dma_start(out=s_t[c], in_=stile[c][:])

                prev1 = None
                prev2 = None
                with nc.allow_low_precision(reason="bf16 edge logits"):
                    for c in range(NCH):
                        for (col0, ncols, g0, ng) in batches[c]:
                            wc0, wcn = col0 * 8, ncols * 8
                            idxsb = bpool.tile([P, BATCH_COLS * 8],
                                               mybir.dt.int16, tag="idx")
                            nc.sync.dma_start(out=idxsb[:, 0:wcn],
                                              in_=idx_t[:, wc0:wc0 + wcn])
                            slots = bpool.tile([P, BATCH_COLS, D2], bf16,
                                               tag="slots")
                            nc.gpsimd.dma_gather(
                                slots[:, 0:ncols, :],
                                table[c * CHROWS:(c + 1) * CHROWS, :],
                                idxsb[:, 0:wcn],
                                ncols * P, ncols * P, D2, single_packet=False)
                            for gi in range(g0, g0 + ng):
                                (b0, nbk, Kg, gcol0) = groups[c][gi]
                                cb = gcol0 - col0
                                nk = nbk * Kg
                                slh = slots[:, cb:cb + nk, HID:D2]
                                sl4 = slots[:, cb:cb + nk, 0:HID].rearrange(
                                    "p (n k) e -> p n k e", k=Kg)
                                tmp = spool.tile([P, nbk, Kg, HID], bf16,
                                                 tag="tmp")
                                nc.vector.tensor_tensor(
                                    out=tmp[:], in0=sl4,
                                    in1=hnD[c][:, b0:b0 + nbk, :].unsqueeze(2)
                                        .to_broadcast([P, nbk, Kg, HID]),
                                    op=mybir.AluOpType.mult)
                                delta = spool.tile([P, nbk, Kg], bf16,
                                                   tag="delta")
                                nc.vector.tensor_reduce(
                                    out=delta[:], in_=tmp[:],
                                    axis=mybir.AxisListType.X,
                                    op=mybir.AluOpType.add)
                                ex2 = spool.tile([P, nbk, Kg, 2], bf16,
                                                 tag="ex2")
                                nc.scalar.activation(
                                    ex2[:],
                                    delta[:].unsqueeze(3)
                                        .to_broadcast([P, nbk, Kg, 2]),
                                    mybir.ActivationFunctionType.Exp,
                                    scale=betasb[:])
                                e = dict(c=c, b0=b0, nbk=nbk, Kg=Kg,
                                         ex2=ex2, slh=slh)
                                if prev1 is not None:
                                    s2(prev1)
                                if prev2 is not None:
                                    s3(prev2)
                                prev2, prev1 = prev1, e
                    if prev1 is not None:
                        s2(prev1)
                    if prev2 is not None:
                        s3(prev2)
                    if prev1 is not None:
                        s3(prev1)

        free_agin()
        free_table()

    nc.compile()
    _split_multi_waits(nc)
    return nc


# ------------------------------------------------------------------- driver
_CACHE = {}


def _get_nc(cfg_key, cfg):
    if cfg_key not in _CACHE:
        _CACHE[cfg_key] = _build_nc(cfg)
    return _CACHE[cfg_key]


def _make_in_maps(inputs, prep):
    x = np.asarray(inputs["x"], dtype=np.float32)
    W1 = np.asarray(inputs["W1"], dtype=np.float32)
    b1 = np.asarray(inputs["b1"], dtype=np.float32).reshape(-1, 1)
    W2 = np.asarray(inputs["W2"], dtype=np.float32)
    b2 = np.asarray(inputs["b2"], dtype=np.float32).reshape(-1, 1)
    beta = np.asarray(inputs["beta"], dtype=np.float32)
    beta128 = np.repeat(beta.reshape(1, 1), P, axis=0).astype(np.float32)
    import ml_dtypes
    bf = ml_dtypes.bfloat16

    in_maps = []
    for p in range(CORES):
        xp = x[p * NP:(p + 1) * NP]
        xpad = np.zeros((NPP, IN), np.float32)
        xpad[:NP] = xp
        in_maps.append({
            "xT": np.ascontiguousarray(xpad.T).astype(bf),
            "idx": prep["idx_all"][p],
            "sidx": prep["sidx_all"][p],
            "W1": W1.astype(bf), "b1": b1,
            "W2": W2.astype(bf), "b2": b2,
            "beta128": beta128,
        })
    return in_maps


def _postprocess(results, prep, inputs):
    Wc = np.asarray(inputs["Wc"], dtype=np.float64)
    bc = np.asarray(inputs["bc"], dtype=np.float64)
    beta = float(np.asarray(inputs["beta"]).reshape(-1)[0])
    expb = math.exp(beta)
    K = prep["Keff"]
    y = np.zeros((N, Wc.shape[1]), np.float32)
    n = np.arange(NP)
    bb, dd = n // P, n % P
    for p in range(CORES):
        res = results[p]
        agg = res["agg"].astype(np.float64)       # [4, 128, 98, 64]
        sdev = res["s"].astype(np.float64)        # [4, 128, 98]
        sagg = res["selfagg"].astype(np.float64)  # [128, 98, 64]
        acc = sagg[dd, bb, :].copy()              # self term, base order
        stot = np.full(NP, expb)
        cntp = prep["cnt"][p * NP:(p + 1) * NP]
        for c in range(NCH):
            pos = prep["pos_pc"][p, c]
            b_, d_ = pos // P, pos % P
            acc += agg[c, d_, b_, :]
            stot += sdev[c, d_, b_] / 2.0 - (K[c, b_] - cntp[:, c])
        y[p * NP:(p + 1) * NP] = ((acc / stot[:, None]) @ Wc + bc).astype(np.float32)
    return y


def get_nc_for_test(inputs):
    prep = _preprocess(np.asarray(inputs["edge_index"]))
    cfg = dict(K=prep["K"], TOTC=prep["TOTC"], groups=prep["groups"], batches=prep["batches"])
    cfg_key = ("v6", tuple(int(k) for k in prep["Keff"].ravel()))
    return _get_nc(cfg_key, cfg)


def kernel(**inputs):
    from concourse.bass_utils import run_bass_kernel_spmd

    prep = _preprocess(np.asarray(inputs["edge_index"]))
    cfg = dict(K=prep["K"], TOTC=prep["TOTC"], groups=prep["groups"], batches=prep["batches"])
    cfg_key = ("v6", tuple(int(k) for k in prep["Keff"].ravel()))
    nc = _get_nc(cfg_key, cfg)
    in_maps = _make_in_maps(inputs, prep)
    res = run_bass_kernel_spmd(nc, in_maps, core_ids=list(range(CORES)))
    return _postprocess(res.results, prep, inputs)


if __name__ == "__main__":
    pass



# revision 9
# speedup vs baseline: 1.1158x; 1.0101x over previous
"""AGNN (AMNet) message-passing kernel for 8 TRN2 NeuronCores.

Design (vs the v1 baseline):
  - Nodes partitioned contiguously across 8 cores (no global sort).
  - Phase 1: input MLP in bf16, 4 blocks (512 nodes) per iteration; packed
    [hn|h] bf16 rows stored to a local HBM table `agin` in partition-major
    layout (node n -> flat row (n%128)*98 + n//128).
  - Phase 2: ONE AllGather replicates the table (addr_space Shared).
  - Per-chunk independent dst orderings: for each of the 4 table chunks
    (int16 gather reach), each core re-sorts its OWN dsts by that chunk's
    in-degree, making the per-(block,chunk) rectangle schedule tight
    (~1.1x padding vs 2.07x for a common ordering).
  - Device outputs per-chunk partial aggregates (fp32) and exp-sums; the
    softmax combine across chunks + self-loop term + classifier run on the
    HOST (they commute with the chunk decomposition).
  - No tile_critical / manual semaphores: tile auto-tracks dma_gather.

kernel() accepts FULL inputs and returns the FULL [N, 2] float32 output.
"""

import math
import os
import sys

sys.path.insert(0, "/opt/trn_rl_repo")

import numpy as np

CORES = 8
P = 128
NCH = 4
N = 100000
NP = N // CORES            # 12500
NB = math.ceil(NP / P)     # 98
NPP = NB * P               # 12544
CHROWS = 2 * NPP           # 25088
ZROW = NPP - 1             # 12543  == (12543%128)*98 + 12543//128
SBB = 8                    # blocks per super-block (one gather each)
GB = SBB                   # psum->sbuf copy group
IN, HID = 128, 64
D2 = 2 * HID
NIT = NB // 4 + (1 if NB % 4 else 0)  # 25 MLP iterations (4 blocks each)

AGG_DVE = bool(os.environ.get("AGG_DVE"))  # A/B: aggregation on DVE vs PE
MAXG = 8                       # max blocks per compute group (PSUM)
MAXGC = 48                     # max columns per compute group (SBUF)
BATCH_COLS = 48                # gather batch size (columns)
CCOL = 330.0                   # DP cost per padded column (ns-ish)
CGRP = 1500.0                  # DP cost per extra group


def _dp_groups(Kc):
    """Consecutive groups (<=MAXG blocks, <=MAXGC cols) minimizing
    sum(len*Kmax*CCOL + CGRP)."""
    nb = len(Kc)
    best = [float("inf")] * (nb + 1)
    prev = [0] * (nb + 1)
    best[0] = 0.0
    for i in range(1, nb + 1):
        kmax = 0
        for L in range(1, MAXG + 1):
            j = i - L
            if j < 0:
                break
            kmax = max(kmax, Kc[j])
            if L * kmax > MAXGC:
                break
            cst = best[j] + L * kmax * CCOL + CGRP
            if cst < best[i]:
                best[i] = cst
                prev[i] = j
    out = []
    i = nb
    while i > 0:
        j = prev[i]
        out.append((j, i - j, int(max(Kc[j:i]))))
        i = j
    return out[::-1]


def _flatrow(n):
    """Local table row of local node id n (partition-major layout)."""
    return (n % P) * NB + n // P


def _wrap_idx(flat):
    n = flat.shape[0]
    w = flat.reshape(n // 16, 16).T
    return np.tile(w, (8, 1)).astype(np.int16)


# ----------------------------------------------------------------- host prep
def _preprocess(edge_index):
    src = np.asarray(edge_index[0], dtype=np.int64)
    dst = np.asarray(edge_index[1], dtype=np.int64)
    chunk_of_src = src // (2 * NP)

    # CSR sorted by (dst, chunk)
    order = np.lexsort((src, chunk_of_src, dst))
    src_s = src[order]
    ch_s = chunk_of_src[order]
    # per (dst, chunk) counts
    cnt = np.zeros((N, NCH), np.int64)
    np.add.at(cnt, (dst, chunk_of_src), 1)
    cnt_cum = np.concatenate(
        [np.zeros((N, 1), np.int64), np.cumsum(cnt, axis=1)], axis=1
    )  # [N, 5]
    deg = cnt.sum(axis=1)
    row_start = np.zeros(N + 1, np.int64)
    np.cumsum(deg, out=row_start[1:])

    # local table row of each src (within its chunk)
    q = src_s // NP
    nloc = src_s % NP
    locrow = (q % 2) * NPP + (nloc % P) * NB + nloc // P  # within-chunk row

    # per-(core, chunk) orderings + per-block maxima
    pos_pc = np.zeros((CORES, NCH, NP), np.int64)   # node -> position
    Kpc = np.zeros((CORES, NCH, NB), np.int64)
    for p in range(CORES):
        lo = p * NP
        cl = cnt[lo:lo + NP]                         # [NP, NCH]
        for c in range(NCH):
            o = np.argsort(-cl[:, c], kind="stable")
            pos = np.empty(NP, np.int64)
            pos[o] = np.arange(NP)
            pos_pc[p, c] = pos
            srt = cl[o, c]
            padded = np.zeros(NPP, np.int64)
            padded[:NP] = srt
            Kpc[p, c] = padded.reshape(NB, P).max(axis=1)
    K = Kpc.max(axis=0)                              # [NCH, NB]
    K = np.maximum(K, 1)

    # flexible uniform-K groups per chunk + gather batches of whole groups
    colstart = np.zeros((NCH, NB), np.int64)
    Keff = np.zeros((NCH, NB), np.int64)
    groups = []   # [NCH] list of (b0, nbk, Kg, col0)  (col0 global)
    batches = []  # [NCH] list of (col0, ncols, g0, ng)
    coff = 0
    for c in range(NCH):
        gs = _dp_groups(list(K[c]))
        glist = []
        bl = []
        bc0, bg0, bnc = coff, 0, 0
        for gi, (b0, nbk, Kg) in enumerate(gs):
            w = nbk * Kg
            if bnc + w > BATCH_COLS and bnc > 0:
                bl.append((bc0, bnc, bg0, gi - bg0))
                bc0, bg0, bnc = coff, gi, 0
            glist.append((b0, nbk, Kg, coff))
            Keff[c, b0:b0 + nbk] = Kg
            for j in range(nbk):
                colstart[c, b0 + j] = coff + j * Kg
            coff += w
            bnc += w
        bl.append((bc0, bnc, bg0, len(gs) - bg0))
        groups.append(glist)
        batches.append(bl)
    TOTC = coff

    # index streams
    idx_all = np.zeros((CORES, P, TOTC * 8), np.int16)
    sidx_all = np.zeros((CORES, NCH, P, NPP // 16), np.int16)
    for p in range(CORES):
        lo = p * NP
        e0, e1 = row_start[lo], row_start[lo + NP]
        d_e = dst[order[e0:e1]] - lo
        c_e = ch_s[e0:e1]
        r_e = locrow[e0:e1]
        # k-rank of each edge within its (dst, chunk) segment
        seg0 = row_start[d_e + lo] - e0 + cnt_cum[d_e + lo, c_e]
        k_e = np.arange(e1 - e0) - seg0
        A = np.full((TOTC, P), ZROW, np.int16)
        pos_e = pos_pc[p, c_e, d_e]
        col_e = colstart[c_e, pos_e // P] + k_e
        A[col_e, pos_e % P] = r_e.astype(np.int16)
        idx_all[p] = _wrap_idx(A.ravel())
        for c in range(NCH):
            o = np.argsort(pos_pc[p, c], kind="stable")  # position -> node
            rows = np.full(NPP, ZROW, np.int64)
            rows[:NP] = _flatrow(o)
            sidx_all[p, c] = _wrap_idx(rows)

    return dict(K=K, Keff=Keff, TOTC=TOTC, groups=groups, batches=batches,
                colstart=colstart,
                pos_pc=pos_pc, cnt=cnt, idx_all=idx_all, sidx_all=sidx_all)


# ------------------------------------------------------------------ builder
def _patch_walrus_args():
    import concourse.bass_utils as bu
    if getattr(bu, "_agnn_dge_patch", False):
        return
    orig = bu.get_walrus_args

    def patched(*a, **k):
        return list(orig(*a, **k)) + [
            "--dge-levels=io,spill_reload,scalar_dynamic_offset,"
            "vector_dynamic_offsets,dst_reduce,transpose",
        ]

    bu.get_walrus_args = patched
    bu._agnn_dge_patch = True


def _split_multi_waits(nc):
    import bass_rust
    import concourse.mybir as mybir

    cnt = 0
    for func in nc.m.functions:
        for block in func.blocks:
            out = []
            for inst in block.instructions:
                si = inst.sync_info
                cap = 2 if isinstance(inst, mybir.InstEventSemaphore) else 1
                if (si is not None and si.on_wait and len(si.on_wait) > cap
                        and inst.engine is not None):
                    waits = list(si.on_wait)
                    for w in waits[cap:]:
                        cnt += 1
                        nop = mybir.InstNoOp(
                            name=f"wsplit{cnt}", engine=inst.engine, ins=[], outs=[]
                        )
                        nop.sync_info = bass_rust.SyncInfo(on_wait=[w], on_update=[])
                        try:
                            nc.register_instruction(nop, overwrite=True)
                        except Exception:
                            pass
                        out.append(nop)
                    inst.sync_info = bass_rust.SyncInfo(
                        on_wait=waits[:cap], on_update=list(si.on_update or [])
                    )
                out.append(inst)
            try:
                block.instructions = out
            except Exception:
                block.instructions.clear()
                block.instructions.extend(out)
    return cnt


def _build_nc(cfg):
    _patch_walrus_args()
    import concourse.bacc as bacc
    import concourse.mybir as mybir
    import concourse.tile as tile
    from concourse.masks import make_identity

    K = cfg["K"]; TOTC = cfg["TOTC"]
    groups = cfg["groups"]; batches = cfg["batches"]

    f32 = mybir.dt.float32
    bf16 = mybir.dt.bfloat16

    nc = bacc.Bacc("TRN2", num_devices=CORES, dynamic_dma_scratch_size=65536)

    xT_t = nc.declare_dram_parameter("xT", [IN, NPP], bf16, isOutput=False)
    idx_t = nc.declare_dram_parameter("idx", [P, TOTC * 8], mybir.dt.int16, isOutput=False)
    sidx_t = nc.declare_dram_parameter("sidx", [NCH, P, NPP // 16], mybir.dt.int16, isOutput=False)
    W1_t = nc.declare_dram_parameter("W1", [IN, HID], bf16, isOutput=False)
    b1_t = nc.declare_dram_parameter("b1", [HID, 1], f32, isOutput=False)
    W2_t = nc.declare_dram_parameter("W2", [HID, HID], bf16, isOutput=False)
    b2_t = nc.declare_dram_parameter("b2", [HID, 1], f32, isOutput=False)
    beta_t = nc.declare_dram_parameter("beta128", [P, 1], f32, isOutput=False)
    agg_t = nc.declare_dram_parameter("agg", [NCH, P, NB, HID], bf16, isOutput=True)
    s_t = nc.declare_dram_parameter("s", [NCH, P, NB], f32, isOutput=True)
    sagg_t = nc.declare_dram_parameter("selfagg", [P, NB, HID], f32, isOutput=True)

    with tile.TileContext(nc) as tc:
        agin, free_agin = tc.tile([P, NB, D2], bf16, space="DRAM", name="agin")
        import contextlib
        _tabctx = contextlib.ExitStack()
        _tabpool = _tabctx.enter_context(
            tc.tile_pool(name="tabpool", space="DRAM", bufs=1))
        table = _tabpool.tile([CORES * NPP, D2], bf16, name="table",
                              addr_space="Shared")

        def free_table():
            _tabctx.close()

        with tc.tile_pool(name="consts", bufs=1) as cpool:
            W1sb = cpool.tile([IN, HID], bf16)
            W2sb = cpool.tile([HID, HID], bf16)
            b1sb = cpool.tile([HID, 1], f32)
            b2sb = cpool.tile([HID, 1], f32)
            betasb = cpool.tile([P, 1], f32)
            sbeta = cpool.tile([P, 1], f32)
            ident = cpool.tile([P, P], f32)
            identb = cpool.tile([P, P], bf16)
            hTD = cpool.tile([P, NB, HID], bf16)
            zrow = cpool.tile([1, 1, D2], bf16)
            hnD = [cpool.tile([P, NB, HID], bf16, name=f"hnD{c}") for c in range(NCH)]

            nc.sync.dma_start(out=W1sb[:], in_=W1_t[:])
            nc.sync.dma_start(out=W2sb[:], in_=W2_t[:])
            nc.sync.dma_start(out=b1sb[:], in_=b1_t[:])
            nc.sync.dma_start(out=b2sb[:], in_=b2_t[:])
            nc.sync.dma_start(out=betasb[:], in_=beta_t[:])
            make_identity(nc, ident[:])
            nc.vector.tensor_copy(identb[:], ident[:])
            nc.scalar.activation(sbeta[:], betasb[:], mybir.ActivationFunctionType.Exp)
            nc.gpsimd.memset(zrow[:], 0)
            stile = [cpool.tile([P, NB], f32, name=f"stile{c}") for c in range(NCH)]

            # ---------------- phase 1: MLP + packed local table ------------
            with (
                tc.tile_pool(name="mlp", bufs=3) as mpool,
                tc.tile_pool(name="mlpp", bufs=2, space="PSUM") as mpp,
            ):
                NIT8 = (NB + 7) // 8
                for it in range(NIT8):
                    nbk = min(8, NB - 8 * it)
                    s0 = it * 8 * P
                    hb = mpool.tile([HID, 8 * P], bf16, tag="hb")
                    for h0 in range(0, nbk * P, 512):
                        hc = min(512, nbk * P - h0)
                        xt = mpool.tile([IN, 512], bf16, tag="xt")
                        nc.sync.dma_start(out=xt[:, 0:hc],
                                          in_=xT_t[:, s0 + h0:s0 + h0 + hc])
                        ps1 = mpp.tile([HID, 512], f32, tag="ps1")
                        nc.tensor.matmul(ps1[:, 0:hc], lhsT=W1sb[:], rhs=xt[:, 0:hc],
                                         start=True, stop=True)
                        r1 = mpool.tile([HID, 512], bf16, tag="r1")
                        nc.scalar.activation(r1[:, 0:hc], ps1[:, 0:hc],
                                             mybir.ActivationFunctionType.Relu,
                                             bias=b1sb[:])
                        ps2 = mpp.tile([HID, 512], f32, tag="ps2")
                        nc.tensor.matmul(ps2[:, 0:hc], lhsT=W2sb[:], rhs=r1[:, 0:hc],
                                         start=True, stop=True)
                        nc.vector.tensor_scalar_add(out=hb[:, h0:h0 + hc],
                                                    in0=ps2[:, 0:hc],
                                                    scalar1=b2sb[:])
                    pst = mpp.tile([P, 8, HID], bf16, tag="pst")
                    for j in range(nbk):
                        nc.tensor.transpose(pst[:, j, :], hb[:, j * P:(j + 1) * P],
                                            identb[0:HID, 0:HID])
                    hts = hTD[:, 8 * it:8 * it + nbk, :]
                    nc.scalar.activation(hts, pst[:, 0:nbk, :],
                                         mybir.ActivationFunctionType.Copy)
                    sq = mpool.tile([P, 8, HID], bf16, tag="sq")
                    nc.vector.tensor_tensor(out=sq[:, 0:nbk, :], in0=hts, in1=hts,
                                            op=mybir.AluOpType.mult)
                    ss = mpool.tile([P, 8], f32, tag="ss")
                    nc.vector.tensor_reduce(out=ss[:, 0:nbk], in_=sq[:, 0:nbk, :],
                                            axis=mybir.AxisListType.X,
                                            op=mybir.AluOpType.add)
                    nrm = mpool.tile([P, 8], f32, tag="nrm")
                    nc.scalar.activation(nrm[:, 0:nbk], ss[:, 0:nbk],
                                         mybir.ActivationFunctionType.Sqrt)
                    nc.vector.tensor_scalar_max(out=nrm[:, 0:nbk], in0=nrm[:, 0:nbk],
                                                scalar1=1e-12)
                    rn = mpool.tile([P, 8], f32, tag="rn")
                    nc.vector.reciprocal(rn[:, 0:nbk], nrm[:, 0:nbk])
                    tabt = mpool.tile([P, 8, D2], bf16, tag="tabt")
                    nc.vector.tensor_tensor(
                        out=tabt[:, 0:nbk, 0:HID], in0=hts,
                        in1=rn[:, 0:nbk].unsqueeze(2).to_broadcast([P, nbk, HID]),
                        op=mybir.AluOpType.mult)
                    nc.vector.tensor_copy(tabt[:, 0:nbk, HID:D2], hts)
                    nc.sync.dma_start(out=agin[:, 8 * it:8 * it + nbk, :],
                                      in_=tabt[:, 0:nbk, :])
            # zero row for gather pads — AFTER the MLP stores (overwrites the
            # fake node at (p=127, b=97) = flat row 12543)
            nc.sync.dma_start(out=agin[P - 1:P, NB - 1:NB, :], in_=zrow[:])

            # self-loop partial (base order): exp(beta) * h
            with tc.tile_pool(name="selfp", bufs=1) as spool_:
                sagg = spool_.tile([P, NB, HID], f32)
                nc.vector.tensor_scalar_mul(out=sagg[:], in0=hTD[:], scalar1=sbeta[:])
                nc.sync.dma_start(out=sagg_t[:], in_=sagg[:])

            # ---------------- phase 2: AllGather the table -----------------
            nc.gpsimd.collective_compute(
                "AllGather",
                mybir.AluOpType.bypass,
                replica_groups=[list(range(CORES))],
                ins=[agin[:].flatten_outer_dims().opt()],
                outs=[table[:].opt()],
            )

            # ---------------- phase 2.5: per-chunk dst tables --------------
            with tc.tile_pool(name="hnprep", bufs=1) as hpool:
                aginflat = agin[:].flatten_outer_dims()
                for c in range(NCH):
                    sxi = hpool.tile([P, NPP // 16], mybir.dt.int16, tag="sxi")
                    nc.sync.dma_start(out=sxi[:], in_=sidx_t[c])
                    scr = hpool.tile([P, NB, D2], bf16, tag="scr")
                    nc.gpsimd.dma_gather(scr[:], aginflat, sxi[:], NPP, NPP, D2,
                                         single_packet=False)
                    nc.vector.tensor_copy(hnD[c][:], scr[:, :, 0:HID])

            # ---------------- phase 3: batched per-group AGNN partials -----
            # Stage-staggered: s1(dots+exp) for group g, s2(weights+PE) for
            # g-1, s3(psum copy+store) for g-2.
            with (
                tc.tile_pool(name="blk", bufs=5) as bpool,
                tc.tile_pool(name="blks", bufs=2) as spool,
                tc.tile_pool(name="sacc", bufs=7) as apool,
                tc.tile_pool(name="blkp", bufs=8, space="PSUM") as bpp,
            ):
                ngr = [len(groups[c]) for c in range(NCH)]
                done = [0] * NCH

                def s2(e):
                    c = e["c"]; nbk = e["nbk"]; Kg = e["Kg"]; nk = nbk * Kg
                    nc.vector.tensor_reduce(
                        out=stile[c][:, e["b0"]:e["b0"] + nbk], in_=e["ex2"][:],
                        axis=mybir.AxisListType.XY, op=mybir.AluOpType.add)
                    wts = spool.tile([P, nbk, Kg, HID], bf16, tag="wts")
                    nc.vector.tensor_tensor(
                        out=wts[:].rearrange("p n k (a b) -> p (n k) a b", b=2),
                        in0=e["slh"].rearrange("p c (a b) -> p c a b", b=2),
                        in1=e["ex2"][:].rearrange("p n k e -> p (n k) e")
                            .unsqueeze(2).to_broadcast([P, nk, 32, 2]),
                        op=mybir.AluOpType.mult)
                    pgrp = bpp.tile([P, MAXG, HID], f32, tag="pgrp")
                    for k in range(Kg):
                        nc.tensor.matmul(pgrp[:, 0:nbk, :], lhsT=identb[:],
                                         rhs=wts[:, :, k, :],
                                         start=(k == 0), stop=(k == Kg - 1))
                    e["pgrp"] = pgrp

                def s3(e):
                    c = e["c"]; nbk = e["nbk"]
                    aggsb = apool.tile([P, MAXG, HID], bf16, tag="aggsb")
                    nc.scalar.activation(aggsb[:, 0:nbk, :], e["pgrp"][:, 0:nbk, :],
                                         mybir.ActivationFunctionType.Copy)
                    nc.sync.dma_start(out=agg_t[c, :, e["b0"]:e["b0"] + nbk, :],
                                      in_=aggsb[:, 0:nbk, :])
                    done[c] += 1
                    if done[c] == ngr[c]:
                        nc.sync.dma_start(out=s_t[c], in_=stile[c][:])

                prev1 = None
                prev2 = None
                with nc.allow_low_precision(reason="bf16 edge logits"):
                    for c in range(NCH):
                        for (col0, ncols, g0, ng) in batches[c]:
                            wc0, wcn = col0 * 8, ncols * 8
                            idxsb = bpool.tile([P, BATCH_COLS * 8],
                                               mybir.dt.int16, tag="idx")
                            nc.sync.dma_start(out=idxsb[:, 0:wcn],
                                              in_=idx_t[:, wc0:wc0 + wcn])
                            slots = bpool.tile([P, BATCH_COLS, D2], bf16,
                                               tag="slots")
                            nc.gpsimd.dma_gather(
                                slots[:, 0:ncols, :],
                                table[c * CHROWS:(c + 1) * CHROWS, :],
                                idxsb[:, 0:wcn],
                                ncols * P, ncols * P, D2, single_packet=False)
                            for gi in range(g0, g0 + ng):
                                (b0, nbk, Kg, gcol0) = groups[c][gi]
                                cb = gcol0 - col0
                                nk = nbk * Kg
                                slh = slots[:, cb:cb + nk, HID:D2]
                                sl4 = slots[:, cb:cb + nk, 0:HID].rearrange(
                                    "p (n k) e -> p n k e", k=Kg)
                                tmp = spool.tile([P, nbk, Kg, HID], bf16,
                                                 tag="tmp")
                                nc.vector.tensor_tensor(
                                    out=tmp[:], in0=sl4,
                                    in1=hnD[c][:, b0:b0 + nbk, :].unsqueeze(2)
                                        .to_broadcast([P, nbk, Kg, HID]),
                                    op=mybir.AluOpType.mult)
                                delta = spool.tile([P, nbk, Kg], bf16,
                                                   tag="delta")
                                nc.vector.tensor_reduce(
                                    out=delta[:], in_=tmp[:],
                                    axis=mybir.AxisListType.X,
                                    op=mybir.AluOpType.add)
                                ex2 = spool.tile([P, nbk, Kg, 2], bf16,
                                                 tag="ex2")
                                nc.scalar.activation(
                                    ex2[:],
                                    delta[:].unsqueeze(3)
                                        .to_broadcast([P, nbk, Kg, 2]),
                                    mybir.ActivationFunctionType.Exp,
                                    scale=betasb[:])
                                e = dict(c=c, b0=b0, nbk=nbk, Kg=Kg,
                                         ex2=ex2, slh=slh)
                                if prev1 is not None:
                                    s2(prev1)
                                if prev2 is not None:
                                    s3(prev2)
                                prev2, prev1 = prev1, e
                    if prev1 is not None:
                        s2(prev1)
                    if prev2 is not None:
                        s3(prev2)
                    if prev1 is not None:
                        s3(prev1)

        free_agin()
        free_table()

    nc.compile()
    _split_multi_waits(nc)
    return nc


# ------------------------------------------------------------------- driver
_CACHE = {}


def _get_nc(cfg_key, cfg):
    if cfg_key not in _CACHE:
        _CACHE[cfg_key] = _build_nc(cfg)
    return _CACHE[cfg_key]


def _make_in_maps(inputs, prep):
    x = np.asarray(inputs["x"], dtype=np.float32)
    W1 = np.asarray(inputs["W1"], dtype=np.float32)
    b1 = np.asarray(inputs["b1"], dtype=np.float32).reshape(-1, 1)
    W2 = np.asarray(inputs["W2"], dtype=np.float32)
    b2 = np.asarray(inputs["b2"], dtype=np.float32).reshape(-1, 1)
    beta = np.asarray(inputs["beta"], dtype=np.float32)
    beta128 = np.repeat(beta.reshape(1, 1), P, axis=0).astype(np.float32)
    import ml_dtypes
    bf = ml_dtypes.bfloat16

    in_maps = []
    for p in range(CORES):
        xp = x[p * NP:(p + 1) * NP]
        xpad = np.zeros((NPP, IN), np.float32)
        xpad[:NP] = xp
        in_maps.append({
            "xT": np.ascontiguousarray(xpad.T).astype(bf),
            "idx": prep["idx_all"][p],
            "sidx": prep["sidx_all"][p],
            "W1": W1.astype(bf), "b1": b1,
            "W2": W2.astype(bf), "b2": b2,
            "beta128": beta128,
        })
    return in_maps


def _postprocess(results, prep, inputs):
    Wc = np.asarray(inputs["Wc"], dtype=np.float64)
    bc = np.asarray(inputs["bc"], dtype=np.float64)
    beta = float(np.asarray(inputs["beta"]).reshape(-1)[0])
    expb = math.exp(beta)
    K = prep["Keff"]
    y = np.zeros((N, Wc.shape[1]), np.float32)
    n = np.arange(NP)
    bb, dd = n // P, n % P
    for p in range(CORES):
        res = results[p]
        agg = res["agg"].astype(np.float64)       # [4, 128, 98, 64]
        sdev = res["s"].astype(np.float64)        # [4, 128, 98]
        sagg = res["selfagg"].astype(np.float64)  # [128, 98, 64]
        acc = sagg[dd, bb, :].copy()              # self term, base order
        stot = np.full(NP, expb)
        cntp = prep["cnt"][p * NP:(p + 1) * NP]
        for c in range(NCH):
            pos = prep["pos_pc"][p, c]
            b_, d_ = pos // P, pos % P
            acc += agg[c, d_, b_, :]
            stot += sdev[c, d_, b_] / 2.0 - (K[c, b_] - cntp[:, c])
        y[p * NP:(p + 1) * NP] = ((acc / stot[:, None]) @ Wc + bc).astype(np.float32)
    return y


def get_nc_for_test(inputs):
    prep = _preprocess(np.asarray(inputs["edge_index"]))
    cfg = dict(K=prep["K"], TOTC=prep["TOTC"], groups=prep["groups"], batches=prep["batches"])
    cfg_key = ("v6", tuple(int(k) for k in prep["Keff"].ravel()))
    return _get_nc(cfg_key, cfg)


def kernel(**inputs):
    from concourse.bass_utils import run_bass_kernel_spmd

    prep = _preprocess(np.asarray(inputs["edge_index"]))
    cfg = dict(K=prep["K"], TOTC=prep["TOTC"], groups=prep["groups"], batches=prep["batches"])
    cfg_key = ("v6", tuple(int(k) for k in prep["Keff"].ravel()))
    nc = _get_nc(cfg_key, cfg)
    in_maps = _make_in_maps(inputs, prep)
    res = run_bass_kernel_spmd(nc, in_maps, core_ids=list(range(CORES)))
    return _postprocess(res.results, prep, inputs)


if __name__ == "__main__":
    pass

